# revision 40
# baseline (speedup 1.0000x reference)
"""ADGN (antisymmetric DGN) message-passing GNN on 8 TRN2 NeuronCores.

Strategy (self-contained, hardcoded for the nn_ADGN problem):
  - Nodes sharded at GRAPH boundaries: core c owns graphs [16c, 16c+16).
    Per-core node counts padded to uniform NPC_PAD (one SPMD program).
  - State HT [128 feat, NPC_PAD] f32 in SBUF; replicated bf16 h table in
    DRAM, QUARTER-MAJOR layout: row = q*2*npc + core*(npc/4) + (local %
    npc/4), q = local quarter. Two copies (A/B) alternate per iteration;
    the 4 quarter-stripes are rebuilt by 4 small AllGathers that fire as
    soon as each quarter's staging rows are written, overlapping the
    collective with the remaining sweep.
  - Iteration = fused sweep over dst-block GROUPS (GB=4 blocks, grouped
    within a table quarter):
      * one dma_gather call per (stream k, dst block b) bucket on SWDGE
        queue k; per-core token counts are exact - pad tokens carry
        index -1 which the Q7 desc-gen trims (no descriptors, no DMA).
      * one-hot tiles generated ON-CHIP, one DVE tensor_tensor is_equal
        per (group, stream) over the contiguous column range:
        oh[p, c, j] = (slot[p, c] == iota[j]).
      * per dst block: PSUM accumulates msg.T @ onehot over all streams'
        chunks; then fused phase B: conv = aW.T@HT_b + lin.T@ACC_b,
        HT_b += eps*tanh(conv+bias), PE transpose -> bf16 staging ->
        DRAM stage (+ pool-sum matmuls in the last iteration).
  - Iter 0 table comes from a full-table node-major embedding computed
    locally on every core (bias via augmented ones-row): no initial
    AllGather; 3 iterations x 4 quarter-AllGathers total.
  - Pooling: sum via batch-one-hot matmuls on staging tiles; max via
    -inf-padded dma_gather + two-level reduce; mean = sum * 1/cnt.
  - Readout MLP on-device per core ([16 graphs, 16]); host concatenates.
"""

import sys

if "/opt/trn_rl_repo" not in sys.path:
    sys.path.insert(0, "/opt/trn_rl_repo")

import numpy as np
import ml_dtypes

import concourse.bacc as bacc
import concourse.bass as bass
import concourse.mybir as mybir
import concourse.tile as tile
from concourse.bass_utils import run_bass_kernel_spmd

F32 = mybir.dt.float32
BF16 = mybir.dt.bfloat16
I16 = mybir.dt.int16

NCORES = 8
N_GRAPHS = 128
GPC = N_GRAPHS // NCORES  # graphs per core
IN_DIM, HID, OUT_DIM = 64, 128, 16
NUM_ITERS = 4
GAMMA, EPS = 0.1, 0.1
NQ = 4                  # SWDGE queues == streams == table quarters
GB = 4                  # dst blocks per group
EMB_CHUNK = 512
NEG_BIG = -1.0e30
POOL_COLS = 14          # max gather-call size for POOLING calls
MSG_BUFS = 20
SUBCH = 8              # chunks per gather subcall
DEFER_K3 = 0            # groups whose stream-3 subcalls are deferred at the
                        # iteration start (hides the last quarter AllGather)


def _ceil_to(x, m):
    return -(-x // m) * m


# ---------------------------------------------------------------- host plan

class Plan:
    pass


def build_plan(edge_index, batch):
    """Host-side preprocessing: shared chunk schedule + per-core arrays."""
    p = Plan()
    n_nodes = batch.shape[0]
    src = np.asarray(edge_index[0], dtype=np.int64)
    dst = np.asarray(edge_index[1], dtype=np.int64)
    batch = np.asarray(batch, dtype=np.int64)
    assert (np.diff(batch) >= 0).all(), "batch must be sorted"

    # graph -> core, node -> core
    graph_start = np.searchsorted(batch, np.arange(N_GRAPHS + 1))  # [129]
    core_start = graph_start[:: GPC]  # [9]
    n_c = np.diff(core_start)
    npc_pad = int(_ceil_to(max(int(n_c.max()), EMB_CHUNK), EMB_CHUNK))
    assert 2 * npc_pad <= 32767
    p.npc_pad = npc_pad
    p.nblk = npc_pad // 128
    p.qs = npc_pad // NQ          # local rows per table quarter
    p.ss = NCORES * p.qs          # table-quarter stripe rows (= 2*npc)
    p.nbq = p.nblk // NQ          # dst blocks per quarter
    p.core_start = core_start
    p.n_c = n_c
    nblk = p.nblk

    node_core = (batch // GPC).astype(np.int64)
    node_local = np.arange(n_nodes, dtype=np.int64) - core_start[node_core]
    # QUARTER-MAJOR shared-table row (stripe = local quarter, then core):
    # lets each quarter's AllGather fire as soon as that quarter is staged
    table_row = ((node_local // p.qs) * p.ss + node_core * p.qs
                 + node_local % p.qs)

    # ---- edge grouping in (quarter, group, k, b) order
    e_tr = table_row[src]
    e_k = e_tr // p.ss                      # stream = src quarter
    e_gidx = (e_tr - e_k * p.ss).astype(np.int16)
    e_core = node_core[dst]
    e_ld = node_local[dst]
    e_b = e_ld // 128
    e_slot = (e_ld % 128).astype(np.int64)

    # schedule: quarters -> groups of GB blocks -> streams -> blocks
    groups = []          # list of (list of blocks)
    for q in range(NQ):
        b0 = q * p.nbq
        for g0 in range(0, p.nbq, GB):
            groups.append(list(range(b0 + g0, b0 + min(g0 + GB, p.nbq))))
    p.groups = groups
    p.ngroups = len(groups)
    p.q_last_group = [max(i for i, g in enumerate(groups)
                          if g[0] // p.nbq == q) for q in range(NQ)]

    # bucket index per (group, k, b-within-group)
    bucket_of = np.full((NQ, nblk), -1, np.int64)   # [k, b] -> bucket id
    bid = 0
    for g in groups:
        for k in range(NQ):
            for b in g:
                bucket_of[k, b] = bid
                bid += 1
    n_bkt = bid
    e_bkt = bucket_of[e_k, e_b]

    key = e_core * n_bkt + e_bkt
    order = np.argsort(key * np.int64(40000) + e_gidx, kind="stable")
    key_s = key[order]
    counts = np.bincount(key_s, minlength=NCORES * n_bkt)
    counts = counts.reshape(NCORES, n_bkt)

    chunk_counts = (-(-counts // 128)).max(axis=0)  # [n_bkt]
    # every block needs >=1 chunk at k=0 so the PSUM start flag exists
    for b in range(nblk):
        j = bucket_of[0, b]
        chunk_counts[j] = max(chunk_counts[j], 1)
    p.chunk_counts = chunk_counts
    p.ccmax = int(chunk_counts.max())
    col_off = np.zeros_like(chunk_counts)
    col_off[1:] = np.cumsum(chunk_counts)[:-1]
    p.col_off = col_off
    p.c_tot = int(chunk_counts.sum())

    # ---- per-group schedule: gather subcalls + one-hot slices
    # p.sched[G] = (oh_list, call_list, subs)
    #   oh_list: per stream k with cols: (k, oh_c0, oh_ncols)
    #   call_list: (k, b, c0, cc) per bucket with cc > 0
    #   subs: gather subcalls (k, sc0, nch), <=SUBCH chunks each, issued
    #         sub-major / stream-minor so the 4 SWDGE queues drain in
    #         parallel (each subcall fits the 1024-desc ring carveout).
    p.sched = []
    p.max_ohcols = 0
    for gi, g in enumerate(groups):
        oh_list = []
        call_list = []
        for k in range(NQ):
            kc0 = None
            kcols = 0
            for b in g:
                j = bucket_of[k, b]
                cc = int(chunk_counts[j])
                if cc == 0:
                    continue
                c0 = int(col_off[j])
                if kc0 is None:
                    kc0 = c0
                assert c0 == kc0 + kcols
                call_list.append((k, b, c0, cc))
                kcols += cc
            if kc0 is not None:
                oh_list.append((k, kc0, kcols))
                p.max_ohcols = max(p.max_ohcols, kcols)
        subs = []
        if oh_list:
            maxs = max(-(-kcols // SUBCH) for (_, _, kcols) in oh_list)
            for s in range(maxs):
                for (k, kc0, kcols) in oh_list:
                    if s * SUBCH < kcols:
                        subs.append((k, kc0 + s * SUBCH,
                                     min(SUBCH, kcols - s * SUBCH)))
        p.sched.append((oh_list, call_list, subs))

    # ---- per-core token data (pads: idx -1 / slot -1)
    tok_tot = p.c_tot * 128
    p.tok_tot = tok_tot
    seg_start = np.zeros(NCORES * n_bkt + 1, np.int64)
    seg_start[1:] = np.cumsum(counts.reshape(-1))
    within = np.arange(len(order), dtype=np.int64) - seg_start[key_s]
    tokpos = col_off[key_s % n_bkt] * 128 + within

    p.msgidx = []
    p.slots = []
    for c in range(NCORES):
        mask = (key_s // n_bkt) == c
        tp = tokpos[mask]
        gi = e_gidx[order][mask]
        sl = e_slot[order][mask]
        arr16 = np.zeros((16, tok_tot // 16), np.int16)
        arr16[tp % 16, tp // 16] = gi
        p.msgidx.append(np.tile(arr16, (8, 1)))
        slots = np.full((128, p.c_tot, 1), -1.0, ml_dtypes.bfloat16)
        slots[tp % 128, tp // 128, 0] = sl.astype(ml_dtypes.bfloat16)
        p.slots.append(slots)

    # ---- pooling
    gsz = np.diff(graph_start)
    p.k_pool = int(_ceil_to(max(int(gsz.max()), 128), 128))
    pool_cols_per_graph = p.k_pool // 128
    graphs_per_call = max(1, POOL_COLS // pool_cols_per_graph)
    p.pool_calls = []
    g = 0
    while g < GPC:
        ng = min(graphs_per_call, GPC - g)
        p.pool_calls.append((g, ng))
        g += ng
    pool_tok = GPC * p.k_pool
    p.pool_tok = pool_tok

    p.poolidx = []
    p.poolhot = []
    p.invcnt = []
    for c in range(NCORES):
        idx = np.full(pool_tok, npc_pad, np.int64)  # default: -inf pad row
        for j in range(GPC):
            g_id = c * GPC + j
            s = graph_start[g_id] - core_start[c]
            e = graph_start[g_id + 1] - core_start[c]
            idx[j * p.k_pool: j * p.k_pool + (e - s)] = np.arange(s, e)
        arr16 = np.zeros((16, pool_tok // 16), np.int16)
        t = np.arange(pool_tok)
        arr16[t % 16, t // 16] = idx.astype(np.int16)
        p.poolidx.append(np.tile(arr16, (8, 1)))

        ph = np.zeros((128, p.nblk, GPC), ml_dtypes.bfloat16)
        ln = np.arange(int(n_c[c]), dtype=np.int64)
        gslot = batch[core_start[c]: core_start[c + 1]] - c * GPC
        ph[ln % 128, ln // 128, gslot] = 1.0
        p.poolhot.append(ph)

        cnt = gsz[c * GPC: (c + 1) * GPC].astype(np.float32)
        inv = 1.0 / np.maximum(cnt, 1.0)
        p.invcnt.append(np.tile(inv[None, :], (128, 1)).astype(np.float32))

    return p


def prepare_inputs(p, x, emb_w, emb_b, W, asym_b, lin_w, r1_w, r1_b, r2_w,
                   r2_b):
    """Build per-core in_maps."""
    aW = W - W.T - GAMMA * np.eye(HID, dtype=np.float32)
    npc = p.npc_pad
    qs = p.qs

    # local feature-major x (embedding B input)
    xT_old = np.zeros((IN_DIM + 1, NCORES * npc), np.float32)
    for c in range(NCORES):
        s, n = int(p.core_start[c]), int(p.n_c[c])
        xT_old[:IN_DIM, c * npc: c * npc + n] = x[s: s + n].T
    xT_old[IN_DIM, :] = 1.0
    # iter-0 gather table: padded-x rows [x | 1 | 0...] in QUARTER-MAJOR
    # row order (col IN_DIM = 1 so aggregation also yields the degree row;
    # emb is applied post-aggregation by linearity: Agg(h0)=emb(Agg([x;1])))
    cm = np.arange(NCORES * npc)
    cc_, ll_ = cm // npc, cm % npc
    qm = (ll_ // qs) * p.ss + cc_ * qs + (ll_ % qs)
    xrows = np.zeros((NCORES * npc, HID), ml_dtypes.bfloat16)
    xrows[qm, : IN_DIM + 1] = xT_old.T.astype(ml_dtypes.bfloat16)

    embWT_aug = np.zeros((IN_DIM + 1, HID), np.float32)
    embWT_aug[:IN_DIM] = emb_w.T
    embWT_aug[IN_DIM] = emb_b

    iota = (np.arange(128, dtype=np.float32)[None, None, :] * np.ones(
        (128, 1, 1), np.float32)).astype(ml_dtypes.bfloat16)

    shared = {
        "embWT": np.ascontiguousarray(embWT_aug),
        "xrows": xrows,
        "iota": np.ascontiguousarray(iota),
        "aWT": np.ascontiguousarray(aW.T.astype(np.float32)),
        "linWT": np.ascontiguousarray(lin_w.T).astype(ml_dtypes.bfloat16),
        "asymB": asym_b.astype(np.float32).reshape(128, 1),
        "ident": np.eye(128, dtype=np.float32),
        "r1wt_add": np.ascontiguousarray(r1_w[:, 0:128].T.astype(np.float32)),
        "r1wt_max": np.ascontiguousarray(r1_w[:, 128:256].T.astype(np.float32)),
        "r1wt_mean": np.ascontiguousarray(r1_w[:, 256:384].T.astype(np.float32)),
        "r1b_a": r1_b[0:128].astype(np.float32).reshape(128, 1),
        "r1b_b": r1_b[128:192].astype(np.float32).reshape(64, 1),
        "r2wt_a": np.ascontiguousarray(r2_w[:, 0:128].T.astype(np.float32)),
        "r2wt_b": np.ascontiguousarray(r2_w[:, 128:192].T.astype(np.float32)),
        "r2b": np.tile(r2_b.astype(np.float32).reshape(1, 16), (GPC, 1)),
    }
    in_maps = []
    for c in range(NCORES):
        m = dict(shared)
        m["xTloc"] = np.ascontiguousarray(xT_old[:, c * npc: (c + 1) * npc])
        m["msgidx"] = p.msgidx[c]
        m["slots"] = p.slots[c]
        m["poolidx"] = p.poolidx[c]
        m["poolhot"] = np.ascontiguousarray(
            p.poolhot[c].reshape(128, p.nblk * GPC))
        m["invcnt"] = p.invcnt[c]
        in_maps.append(m)
    return in_maps


# ---------------------------------------------------------------- device

def build_program(p, stage=99):
    nc = bacc.Bacc("TRN2", num_devices=NCORES, num_swdge_queues=NQ,
                   debug=False)
    npc = p.npc_pad
    nblk = p.nblk
    qrows = npc // NQ  # staging rows per quarter

    # I/O
    d_xrows = nc.dram_tensor("xrows", [NCORES * npc, HID], BF16,
                             kind="ExternalInput")
    d_xTloc = nc.dram_tensor("xTloc", [IN_DIM + 1, npc], F32,
                             kind="ExternalInput")
    d_embWT = nc.dram_tensor("embWT", [IN_DIM + 1, HID], F32,
                             kind="ExternalInput")
    d_iota = nc.dram_tensor("iota", [128, 1, 128], BF16, kind="ExternalInput")
    d_aWT = nc.dram_tensor("aWT", [HID, HID], F32, kind="ExternalInput")
    d_linWT = nc.dram_tensor("linWT", [HID, HID], BF16, kind="ExternalInput")
    d_asymB = nc.dram_tensor("asymB", [HID, 1], F32, kind="ExternalInput")
    d_ident = nc.dram_tensor("ident", [128, 128], F32, kind="ExternalInput")
    d_msgidx = nc.dram_tensor("msgidx", [128, p.tok_tot // 16], I16,
                              kind="ExternalInput")
    d_slots = nc.dram_tensor("slots", [128, p.c_tot, 1], BF16,
                             kind="ExternalInput")
    d_poolidx = nc.dram_tensor("poolidx", [128, p.pool_tok // 16], I16,
                               kind="ExternalInput")
    d_poolhot = nc.dram_tensor("poolhot", [128, nblk * GPC], BF16,
                               kind="ExternalInput")
    d_invcnt = nc.dram_tensor("invcnt", [128, GPC], F32, kind="ExternalInput")
    d_r1wt = [nc.dram_tensor(nm, [128, 192], F32, kind="ExternalInput")
              for nm in ("r1wt_add", "r1wt_max", "r1wt_mean")]
    d_r1b_a = nc.dram_tensor("r1b_a", [128, 1], F32, kind="ExternalInput")
    d_r1b_b = nc.dram_tensor("r1b_b", [64, 1], F32, kind="ExternalInput")
    d_r2wt_a = nc.dram_tensor("r2wt_a", [128, 16], F32, kind="ExternalInput")
    d_r2wt_b = nc.dram_tensor("r2wt_b", [64, 16], F32, kind="ExternalInput")
    d_r2b = nc.dram_tensor("r2b", [GPC, 16], F32, kind="ExternalInput")
    d_out = nc.dram_tensor("out", [GPC, OUT_DIM], F32, kind="ExternalOutput")
    d_dbg = nc.dram_tensor("dbg", [128, 1024], F32, kind="ExternalOutput")

    # internal DRAM
    d_stage = nc.dram_tensor("stage", [npc + 128, HID], BF16, kind="Internal")
    d_tab = [nc.dram_tensor(f"table{t}", [NCORES * npc, HID], BF16,
                            kind="Internal", addr_space="Shared")
             for t in range(2)]

    Tanh = mybir.ActivationFunctionType.Tanh
    Ident = mybir.ActivationFunctionType.Identity
    ADD = mybir.AluOpType.add
    MAX = mybir.AluOpType.max
    MULT = mybir.AluOpType.mult
    ISEQ = mybir.AluOpType.is_equal
    X = mybir.AxisListType.X
    rg = [list(range(NCORES))]

    with tile.TileContext(nc) as tc:
        with tc.tile_pool(name="const", bufs=1) as cst, \
             tc.tile_pool(name="state", bufs=1) as st, \
             tc.tile_pool(name="msg", bufs=MSG_BUFS) as msgp, \
             tc.tile_pool(name="pmsg", bufs=2) as pmsgp, \
             tc.tile_pool(name="oh", bufs=2) as ohp, \
             tc.tile_pool(name="idx", bufs=2) as idxp, \
             tc.tile_pool(name="wrk", bufs=4) as wrk, \
             tc.tile_pool(name="xt", bufs=2) as xtp, \
             tc.tile_pool(name="stg", bufs=2) as stgp, \
             tc.tile_pool(name="psA", bufs=GB, space="PSUM") as psA, \
             tc.tile_pool(name="psB", bufs=2, space="PSUM") as psB, \
             tc.tile_pool(name="psT", bufs=1, space="PSUM") as psT, \
             tc.tile_pool(name="psP", bufs=1, space="PSUM") as psP:

            # ---- load constants
            embWT = cst.tile([IN_DIM + 1, HID], F32)
            iota = cst.tile([128, 1, 128], BF16)
            aWT = cst.tile([HID, HID], F32)
            linWT = cst.tile([HID, HID], BF16)
            asymB = cst.tile([HID, 1], F32)
            ident = cst.tile([128, 128], F32)
            poolhot = cst.tile([128, nblk * GPC], BF16)
            invcnt = cst.tile([128, GPC], F32)
            slots = cst.tile([128, p.c_tot, 1], BF16)
            nc.sync.dma_start(embWT[:], d_embWT[:])
            nc.sync.dma_start(iota[:], d_iota[:])
            nc.sync.dma_start(aWT[:], d_aWT[:])
            nc.sync.dma_start(linWT[:], d_linWT[:])
            nc.sync.dma_start(asymB[:], d_asymB[:])
            nc.sync.dma_start(ident[:], d_ident[:])
            nc.sync.dma_start(poolhot[:], d_poolhot[:])
            nc.sync.dma_start(invcnt[:], d_invcnt[:])
            nc.scalar.dma_start(slots[:], d_slots[:])

            HT = st.tile([HID, npc], F32)
            ACC = st.tile([HID, npc], BF16)
            idx_res = st.tile([128, p.tok_tot // 16], I16)
            nc.scalar.dma_start(idx_res[:], d_msgidx[:])

            # -inf pad row for max-pool gather
            minf = wrk.tile([1, HID], BF16, tag="minf")
            nc.vector.memset(minf[:], NEG_BIG)
            nc.sync.dma_start(d_stage[npc: npc + 1, :], minf[:])

            # (no embedding-A table build: iter 0 gathers padded-x rows
            # directly and applies emb post-aggregation, by linearity)
            embWT16 = cst.tile([IN_DIM + 1, HID], BF16)
            nc.scalar.copy(embWT16[:], embWT[:])

            # ---- embedding B: local feature-major HT
            for j in range(npc // EMB_CHUNK):
                xt = xtp.tile([IN_DIM + 1, EMB_CHUNK], F32, tag="xtl")
                nc.scalar.dma_start(
                    xt[:], d_xTloc[:, j * EMB_CHUNK: (j + 1) * EMB_CHUNK])
                pse = psB.tile([HID, EMB_CHUNK], F32, tag="conv")
                nc.tensor.matmul(pse[:], embWT[:], xt[:], start=True,
                                 stop=True)
                nc.vector.tensor_copy(
                    HT[:, j * EMB_CHUNK: (j + 1) * EMB_CHUNK], pse[:])

            # ---------------- main iterations
            pool_ps = None
            n_iters = NUM_ITERS if stage >= 4 else (1 if stage == 3 else 0)
            for it in range(n_iters):
                last = it == n_iters - 1
                if last:
                    pool_ps = psP.tile([HID, GPC], F32, tag="poolps")
                src_tab = d_xrows if it == 0 else d_tab[it % 2]
                # gather emission order: queue-rotated subcalls; the first
                # DEFER_K3 groups' stream-3 subcalls are pushed behind the
                # other streams so the Pool engine keeps gathering while the
                # previous iteration's last-quarter AllGather lands
                defer = DEFER_K3 if it > 0 else 0
                gorder = []
                stash = []
                for G, (_, _, subs) in enumerate(p.sched):
                    for sub in subs:
                        if G < defer and sub[0] == NQ - 1:
                            stash.append((G,) + sub)
                        else:
                            gorder.append((G,) + sub)
                    if G == defer - 1:
                        gorder += stash
                        stash = []
                last_pos = {}
                for i, e in enumerate(gorder):
                    last_pos[e[0]] = i
                msub_all = [dict() for _ in range(p.ngroups)]
                gpos = 0

                for G, (oh_list, call_list, subs) in enumerate(p.sched):
                    while gpos <= last_pos[G]:
                        (eg, k, sc0, nch) = gorder[gpos]
                        gpos += 1
                        mt = msgp.tile([128, SUBCH, HID], BF16, tag="msg")
                        nc.gpsimd.dma_gather(
                            mt[:, :nch, :],
                            src_tab[k * p.ss: (k + 1) * p.ss, :],
                            idx_res[:, sc0 * 8: (sc0 + nch) * 8],
                            nch * 128, nch * 128, HID,
                            single_packet=True, queue_num=0)
                        for j in range(nch):
                            msub_all[eg][sc0 + j] = (mt, j)
                    msub = msub_all[G]

                    # one-hot per stream over the group's contiguous cols
                    oht = {}
                    for (k, oc0, oncols) in oh_list:
                        oh = ohp.tile([128, p.max_ohcols, 128], BF16,
                                      tag="oh")
                        nc.vector.tensor_tensor(
                            oh[:, :oncols, :],
                            slots[:, oc0: oc0 + oncols, :]
                            .to_broadcast([128, oncols, 128]),
                            iota[:].to_broadcast([128, oncols, 128]),
                            ISEQ)
                        oht[oc0] = (oh, oncols)

                    def oh_slice(c0, j):
                        for oc0, (oh, oncols) in oht.items():
                            if oc0 <= c0 and c0 - oc0 < oncols:
                                return oh, c0 - oc0 + j
                        raise AssertionError

                    # aggregation matmuls (stream order: k=0 starts)
                    blocks = p.groups[G]
                    psb = {}
                    for b in blocks:
                        agg_ps = psA.tile([HID, 128], F32, tag="agg")
                        psb[b] = agg_ps
                    for (k, b, c0, cc) in call_list:
                        for j in range(cc):
                            oh, col = oh_slice(c0, j)
                            mt, mcol = msub[c0 + j]
                            nc.tensor.matmul(
                                psb[b][:], mt[:, mcol, :], oh[:, col, :],
                                start=(k == 0 and j == 0),
                                stop=(k == NQ - 1 and j == cc - 1),
                                skip_group_check=True)

                    # fused phase B
                    stg = stgp.tile([128, GB * HID], BF16, tag="stage")
                    for bi, b in enumerate(blocks):
                        sl = ACC[:, b * 128: (b + 1) * 128]
                        if it == 0:
                            # psb holds Agg([x|1]); apply emb now (deg row
                            # via the ones column folds in the bias term)
                            ax = wrk.tile([IN_DIM + 1, 128], BF16,
                                          tag="aggx")
                            nc.scalar.copy(ax[:], psb[b][0: IN_DIM + 1, :])
                            psE = psB.tile([HID, 128], F32, tag="conv")
                            nc.tensor.matmul(psE[:], embWT16[:], ax[:],
                                             start=True, stop=True)
                            nc.scalar.copy(sl, psE[:])
                        else:
                            nc.scalar.copy(sl, psb[b][:])
                        ps2 = psB.tile([HID, 128], F32, tag="conv")
                        nc.tensor.matmul(ps2[:], aWT[:],
                                         HT[:, b * 128: (b + 1) * 128],
                                         start=True, stop=False)
                        nc.tensor.matmul(ps2[:], linWT[:], sl,
                                         start=False, stop=True)
                        th = wrk.tile([HID, 128], F32, tag="tanh")
                        nc.scalar.activation(th[:], ps2[:], Tanh,
                                             bias=asymB[:])
                        hsl = HT[:, b * 128: (b + 1) * 128]
                        nc.vector.scalar_tensor_tensor(hsl, th[:], EPS, hsl,
                                                       MULT, ADD)
                        trp = psT.tile([128, HID], F32, tag="tr")
                        nc.tensor.transpose(trp[:], hsl, ident[:])
                        nc.scalar.copy(
                            stg[:, bi * HID: (bi + 1) * HID], trp[:])
                        if last:
                            nc.tensor.matmul(
                                pool_ps[:], stg[:, bi * HID: (bi + 1) * HID],
                                poolhot[:, b * GPC: (b + 1) * GPC],
                                start=(b == 0), stop=(b == nblk - 1),
                                skip_group_check=True)
                    b0 = blocks[0]
                    nbG = len(blocks)
                    nc.sync.dma_start(
                        d_stage[b0 * 128: b0 * 128 + nbG * 128, :]
                        .rearrange("(a p) f -> p a f", p=128),
                        stg[:, : nbG * HID].rearrange(
                            "p (a f) -> p a f", f=HID))

                    # quarter q fully staged -> AllGather its table stripe
                    # into the NEXT iteration's table, overlapping the
                    # collective with the remaining groups' gathers (which
                    # read the CURRENT table copy).
                    if not last and G in p.q_last_group:
                        q = p.q_last_group.index(G)
                        nc.gpsimd.collective_compute(
                            "AllGather", mybir.AluOpType.bypass,
                            replica_groups=rg,
                            ins=[d_stage[q * p.qs: (q + 1) * p.qs, :].opt()],
                            outs=[d_tab[(it + 1) % 2]
                                  [q * p.ss: (q + 1) * p.ss, :].opt()])

            # ---------------- debug dumps for staged runs
            if stage < 6:
                w = min(512, npc)
                dbg_t = wrk.tile([128, 1024], F32, tag="dbg")
                nc.vector.memset(dbg_t[:], 0.0)
                if stage >= 3 and n_iters > 0:
                    nc.vector.tensor_copy(dbg_t[:, 0:w], ACC[:, 0:w])
                    nc.vector.tensor_copy(dbg_t[:, 512:512 + w], HT[:, 0:w])
                else:
                    nc.vector.tensor_copy(dbg_t[:, 0:w], HT[:, 0:w])
                nc.sync.dma_start(d_dbg[:], dbg_t[:])

            if stage >= 6:
                # ---------------- pooling
                poolsum = wrk.tile([HID, GPC], F32, tag="psum_sb")
                nc.vector.tensor_copy(poolsum[:], pool_ps[:])
                poolmean = wrk.tile([HID, GPC], F32, tag="pmean_sb")
                nc.vector.tensor_tensor(poolmean[:], poolsum[:], invcnt[:],
                                        MULT)
                poolmax = wrk.tile([HID, GPC], F32, tag="pmax_sb")

                cols_per_g = p.k_pool // 128
                for (g0, ng) in p.pool_calls:
                    ncols = ng * cols_per_g
                    ntok = ncols * 128
                    t0 = g0 * p.k_pool
                    idxt = idxp.tile([128, POOL_COLS * 8], I16, tag="idx")
                    nc.sync.dma_start(
                        idxt[:, : ncols * 8],
                        d_poolidx[:, t0 // 16: t0 // 16 + ncols * 8])
                    gat = pmsgp.tile([128, POOL_COLS, HID], BF16, tag="pmsg")
                    nc.gpsimd.dma_gather(
                        gat[:, :ncols, :], d_stage[:, :],
                        idxt[:, : ncols * 8], ntok, ntok, HID,
                        single_packet=False, queue_num=(g0 // 2) % NQ)
                    for j in range(ng):
                        g = g0 + j
                        part = wrk.tile([128, HID], F32, tag="mpart")
                        nc.vector.tensor_reduce(
                            part[:],
                            gat[:, j * cols_per_g: (j + 1) * cols_per_g, :]
                            .rearrange("p c f -> p f c"),
                            X, MAX)
                        trp = psT.tile([128, HID], F32, tag="tr")
                        nc.tensor.transpose(trp[:], part[:], ident[:])
                        nc.vector.tensor_reduce(
                            poolmax[:, g: g + 1], trp[:], X, MAX)

                # ---------------- readout MLP
                r1wt = []
                for d in d_r1wt:
                    t = cst.tile([128, 192], F32)
                    nc.sync.dma_start(t[:], d[:])
                    r1wt.append(t)
                r1b_a = cst.tile([128, 1], F32)
                r1b_b = cst.tile([64, 1], F32)
                r2wt_a = cst.tile([128, 16], F32)
                r2wt_b = cst.tile([64, 16], F32)
                r2bb = cst.tile([GPC, 16], F32)
                nc.sync.dma_start(r1b_a[:], d_r1b_a[:])
                nc.sync.dma_start(r1b_b[:], d_r1b_b[:])
                nc.sync.dma_start(r2wt_a[:], d_r2wt_a[:])
                nc.sync.dma_start(r2wt_b[:], d_r2wt_b[:])
                nc.sync.dma_start(r2bb[:], d_r2b[:])

                g1 = []
                for (m0, msz, bt) in ((0, 128, r1b_a), (128, 64, r1b_b)):
                    psr = psB.tile([msz, GPC], F32, tag="conv")
                    for wi, src_t in ((0, poolsum), (1, poolmax),
                                      (2, poolmean)):
                        nc.tensor.matmul(psr[:], r1wt[wi][:, m0: m0 + msz],
                                         src_t[:], start=(wi == 0),
                                         stop=(wi == 2))
                    gt = wrk.tile([msz, GPC], F32, tag=f"g1_{m0}")
                    nc.scalar.activation(gt[:], psr[:], Ident, bias=bt[:])
                    nc.vector.scalar_tensor_tensor(gt[:], gt[:], 0.01, gt[:],
                                                   MULT, MAX)
                    g1.append(gt)

                ps2a = psB.tile([GPC, OUT_DIM], F32, tag="conv")
                nc.tensor.matmul(ps2a[:], g1[0][:, :], r2wt_a[:],
                                 start=True, stop=True)
                ps2b = psT.tile([GPC, OUT_DIM], F32, tag="tr")
                nc.tensor.matmul(ps2b[:], g1[1][:, :], r2wt_b[:],
                                 start=True, stop=True)
                t2a = wrk.tile([GPC, OUT_DIM], F32, tag="t2a")
                nc.scalar.copy(t2a[:], ps2a[:])
                t2b = wrk.tile([GPC, OUT_DIM], F32, tag="t2b")
                nc.vector.tensor_tensor(t2b[:], ps2b[:], t2a[:], ADD)
                outt = wrk.tile([GPC, OUT_DIM], F32, tag="outt")
                nc.vector.tensor_tensor(outt[:], t2b[:], r2bb[:], ADD)
                nc.vector.scalar_tensor_tensor(outt[:], outt[:], 0.01,
                                               outt[:], MULT, MAX)
                nc.sync.dma_start(d_out[:], outt[:])

    nc.compile()
    return nc


# ---------------------------------------------------------------- entry

_CACHE = {}


def _run(inputs, trace=False, stage=99):
    x = np.asarray(inputs["x"], np.float32)
    edge_index = np.asarray(inputs["edge_index"])
    batch = np.asarray(inputs["batch"])
    plan_key = (edge_index.tobytes(), batch.tobytes(), stage)
    key = hash(plan_key)
    if key in _CACHE:
        p, nc = _CACHE[key]
    else:
        p = build_plan(edge_index, batch)
        nc = build_program(p, stage=stage)
        _CACHE[key] = (p, nc)

    in_maps = prepare_inputs(
        p, x,
        np.asarray(inputs["emb_w"], np.float32),
        np.asarray(inputs["emb_b"], np.float32),
        np.asarray(inputs["W"], np.float32),
        np.asarray(inputs["asym_b"], np.float32),
        np.asarray(inputs["lin_w"], np.float32),
        np.asarray(inputs["r1_w"], np.float32),
        np.asarray(inputs["r1_b"], np.float32),
        np.asarray(inputs["r2_w"], np.float32),
        np.asarray(inputs["r2_b"], np.float32),
    )
    res = run_bass_kernel_spmd(nc, in_maps, core_ids=list(range(NCORES)),
                               trace=trace)
    out = np.concatenate([res.results[c]["out"] for c in range(NCORES)], 0)
    return out.astype(np.float32), res


def kernel(**inputs):
    out, _ = _run(inputs, trace=False)
    return out



# revision 41
# speedup vs baseline: 2.6979x; 2.6979x over previous
"""ADGN (antisymmetric DGN) message-passing GNN on 8 TRN2 NeuronCores.

Strategy (self-contained, hardcoded for the nn_ADGN problem):
  - Nodes sharded at GRAPH boundaries: core c owns graphs [16c, 16c+16).
    Per-core node counts padded to uniform NPC_PAD (one SPMD program).
  - State HT [128 feat, NPC_PAD] f32 in SBUF; replicated bf16 h table in
    DRAM, QUARTER-MAJOR layout: row = q*2*npc + core*(npc/4) + (local %
    npc/4), q = local quarter. Two copies (A/B) alternate per iteration;
    the 4 quarter-stripes are rebuilt by 4 small AllGathers that fire as
    soon as each quarter's staging rows are written, overlapping the
    collective with the remaining sweep.
  - Iteration = fused sweep over dst-block GROUPS (GB=4 blocks, grouped
    within a table quarter):
      * one dma_gather call per (stream k, dst block b) bucket on SWDGE
        queue k; per-core token counts are exact - pad tokens carry
        index -1 which the Q7 desc-gen trims (no descriptors, no DMA).
      * one-hot tiles generated ON-CHIP, one DVE tensor_tensor is_equal
        per (group, stream) over the contiguous column range:
        oh[p, c, j] = (slot[p, c] == iota[j]).
      * per dst block: PSUM accumulates msg.T @ onehot over all streams'
        chunks; then fused phase B: conv = aW.T@HT_b + lin.T@ACC_b,
        HT_b += eps*tanh(conv+bias), PE transpose -> bf16 staging ->
        DRAM stage (+ pool-sum matmuls in the last iteration).
  - Iter 0 table comes from a full-table node-major embedding computed
    locally on every core (bias via augmented ones-row): no initial
    AllGather; 3 iterations x 4 quarter-AllGathers total.
  - Pooling: sum via batch-one-hot matmuls on staging tiles; max via
    -inf-padded dma_gather + two-level reduce; mean = sum * 1/cnt.
  - Readout MLP on-device per core ([16 graphs, 16]); host concatenates.
"""

import sys

if "/opt/trn_rl_repo" not in sys.path:
    sys.path.insert(0, "/opt/trn_rl_repo")

import numpy as np
import ml_dtypes

import concourse.bacc as bacc
import concourse.bass as bass
import concourse.mybir as mybir
import concourse.tile as tile
from concourse.bass_utils import run_bass_kernel_spmd

F32 = mybir.dt.float32
BF16 = mybir.dt.bfloat16
I16 = mybir.dt.int16

NCORES = 8
N_GRAPHS = 128
GPC = N_GRAPHS // NCORES  # graphs per core
IN_DIM, HID, OUT_DIM = 64, 128, 16
NUM_ITERS = 4
GAMMA, EPS = 0.1, 0.1
NQ = 4                  # SWDGE queues == streams == table quarters
GB = 4                  # dst blocks per group
EMB_CHUNK = 512
NEG_BIG = -1.0e30
POOL_COLS = 14          # max gather-call size for POOLING calls
MSG_BUFS = 20
SUBCH = 4              # chunks per gather subcall
DEFER_K3 = 0            # groups whose stream-3 subcalls are deferred at the
                        # iteration start (hides the last quarter AllGather)


def _ceil_to(x, m):
    return -(-x // m) * m


# ---------------------------------------------------------------- host plan

class Plan:
    pass


def build_plan(edge_index, batch):
    """Host-side preprocessing: shared chunk schedule + per-core arrays."""
    p = Plan()
    n_nodes = batch.shape[0]
    src = np.asarray(edge_index[0], dtype=np.int64)
    dst = np.asarray(edge_index[1], dtype=np.int64)
    batch = np.asarray(batch, dtype=np.int64)
    assert (np.diff(batch) >= 0).all(), "batch must be sorted"

    # graph -> core, node -> core
    graph_start = np.searchsorted(batch, np.arange(N_GRAPHS + 1))  # [129]
    core_start = graph_start[:: GPC]  # [9]
    n_c = np.diff(core_start)
    npc_pad = int(_ceil_to(max(int(n_c.max()), EMB_CHUNK), EMB_CHUNK))
    assert 2 * npc_pad <= 32767
    p.npc_pad = npc_pad
    p.nblk = npc_pad // 128
    p.qs = npc_pad // NQ          # local rows per table quarter
    p.ss = NCORES * p.qs          # table-quarter stripe rows (= 2*npc)
    p.nbq = p.nblk // NQ          # dst blocks per quarter
    p.core_start = core_start
    p.n_c = n_c
    nblk = p.nblk

    node_core = (batch // GPC).astype(np.int64)
    node_local = np.arange(n_nodes, dtype=np.int64) - core_start[node_core]
    # QUARTER-MAJOR shared-table row (stripe = local quarter, then core):
    # lets each quarter's AllGather fire as soon as that quarter is staged
    table_row = ((node_local // p.qs) * p.ss + node_core * p.qs
                 + node_local % p.qs)

    # ---- edge grouping in (quarter, group, k, b) order
    e_tr = table_row[src]
    e_k = e_tr // p.ss                      # stream = src quarter
    e_gidx = (e_tr - e_k * p.ss).astype(np.int16)
    e_core = node_core[dst]
    e_ld = node_local[dst]
    e_b = e_ld // 128
    e_slot = (e_ld % 128).astype(np.int64)

    # schedule: quarters -> groups of GB blocks -> streams -> blocks
    groups = []          # list of (list of blocks)
    for q in range(NQ):
        b0 = q * p.nbq
        for g0 in range(0, p.nbq, GB):
            groups.append(list(range(b0 + g0, b0 + min(g0 + GB, p.nbq))))
    p.groups = groups
    p.ngroups = len(groups)
    p.q_last_group = [max(i for i, g in enumerate(groups)
                          if g[0] // p.nbq == q) for q in range(NQ)]

    # bucket index per (group, k, b-within-group)
    bucket_of = np.full((NQ, nblk), -1, np.int64)   # [k, b] -> bucket id
    bid = 0
    for g in groups:
        for k in range(NQ):
            for b in g:
                bucket_of[k, b] = bid
                bid += 1
    n_bkt = bid
    e_bkt = bucket_of[e_k, e_b]

    key = e_core * n_bkt + e_bkt
    order = np.argsort(key * np.int64(40000) + e_gidx, kind="stable")
    key_s = key[order]
    counts = np.bincount(key_s, minlength=NCORES * n_bkt)
    counts = counts.reshape(NCORES, n_bkt)

    chunk_counts = (-(-counts // 128)).max(axis=0)  # [n_bkt]
    # every block needs >=1 chunk at k=0 so the PSUM start flag exists
    for b in range(nblk):
        j = bucket_of[0, b]
        chunk_counts[j] = max(chunk_counts[j], 1)
    p.chunk_counts = chunk_counts
    p.ccmax = int(chunk_counts.max())
    col_off = np.zeros_like(chunk_counts)
    col_off[1:] = np.cumsum(chunk_counts)[:-1]
    p.col_off = col_off
    p.c_tot = int(chunk_counts.sum())

    # ---- per-group schedule: gather subcalls + one-hot slices
    # p.sched[G] = (oh_list, call_list, subs)
    #   oh_list: per stream k with cols: (k, oh_c0, oh_ncols)
    #   call_list: (k, b, c0, cc) per bucket with cc > 0
    #   subs: gather subcalls (k, sc0, nch), <=SUBCH chunks each, issued
    #         sub-major / stream-minor so the 4 SWDGE queues drain in
    #         parallel (each subcall fits the 1024-desc ring carveout).
    p.sched = []
    p.max_ohcols = 0
    for gi, g in enumerate(groups):
        oh_list = []
        call_list = []
        for k in range(NQ):
            kc0 = None
            kcols = 0
            for b in g:
                j = bucket_of[k, b]
                cc = int(chunk_counts[j])
                if cc == 0:
                    continue
                c0 = int(col_off[j])
                if kc0 is None:
                    kc0 = c0
                assert c0 == kc0 + kcols
                call_list.append((k, b, c0, cc))
                kcols += cc
            if kc0 is not None:
                oh_list.append((k, kc0, kcols))
                p.max_ohcols = max(p.max_ohcols, kcols)
        subs = []
        if oh_list:
            maxs = max(-(-kcols // SUBCH) for (_, _, kcols) in oh_list)
            for s in range(maxs):
                for (k, kc0, kcols) in oh_list:
                    if s * SUBCH < kcols:
                        subs.append((k, kc0 + s * SUBCH,
                                     min(SUBCH, kcols - s * SUBCH)))
        p.sched.append((oh_list, call_list, subs))

    # ---- per-core token data (pads: idx -1 / slot -1)
    tok_tot = p.c_tot * 128
    p.tok_tot = tok_tot
    seg_start = np.zeros(NCORES * n_bkt + 1, np.int64)
    seg_start[1:] = np.cumsum(counts.reshape(-1))
    within = np.arange(len(order), dtype=np.int64) - seg_start[key_s]
    tokpos = col_off[key_s % n_bkt] * 128 + within

    p.msgidx = []
    p.slots = []
    for c in range(NCORES):
        mask = (key_s // n_bkt) == c
        tp = tokpos[mask]
        gi = e_gidx[order][mask]
        sl = e_slot[order][mask]
        arr16 = np.zeros((16, tok_tot // 16), np.int16)
        arr16[tp % 16, tp // 16] = gi
        p.msgidx.append(np.tile(arr16, (8, 1)))
        slots = np.full((128, p.c_tot, 1), -1.0, ml_dtypes.bfloat16)
        slots[tp % 128, tp // 128, 0] = sl.astype(ml_dtypes.bfloat16)
        p.slots.append(slots)

    # ---- pooling
    gsz = np.diff(graph_start)
    p.k_pool = int(_ceil_to(max(int(gsz.max()), 128), 128))
    pool_cols_per_graph = p.k_pool // 128
    graphs_per_call = max(1, POOL_COLS // pool_cols_per_graph)
    p.pool_calls = []
    g = 0
    while g < GPC:
        ng = min(graphs_per_call, GPC - g)
        p.pool_calls.append((g, ng))
        g += ng
    pool_tok = GPC * p.k_pool
    p.pool_tok = pool_tok

    p.poolidx = []
    p.poolhot = []
    p.invcnt = []
    for c in range(NCORES):
        idx = np.full(pool_tok, npc_pad, np.int64)  # default: -inf pad row
        for j in range(GPC):
            g_id = c * GPC + j
            s = graph_start[g_id] - core_start[c]
            e = graph_start[g_id + 1] - core_start[c]
            idx[j * p.k_pool: j * p.k_pool + (e - s)] = np.arange(s, e)
        arr16 = np.zeros((16, pool_tok // 16), np.int16)
        t = np.arange(pool_tok)
        arr16[t % 16, t // 16] = idx.astype(np.int16)
        p.poolidx.append(np.tile(arr16, (8, 1)))

        ph = np.zeros((128, p.nblk, GPC), ml_dtypes.bfloat16)
        ln = np.arange(int(n_c[c]), dtype=np.int64)
        gslot = batch[core_start[c]: core_start[c + 1]] - c * GPC
        ph[ln % 128, ln // 128, gslot] = 1.0
        p.poolhot.append(ph)

        cnt = gsz[c * GPC: (c + 1) * GPC].astype(np.float32)
        inv = 1.0 / np.maximum(cnt, 1.0)
        p.invcnt.append(np.tile(inv[None, :], (128, 1)).astype(np.float32))

    return p


def prepare_inputs(p, x, emb_w, emb_b, W, asym_b, lin_w, r1_w, r1_b, r2_w,
                   r2_b):
    """Build per-core in_maps."""
    aW = W - W.T - GAMMA * np.eye(HID, dtype=np.float32)
    npc = p.npc_pad
    qs = p.qs

    # local feature-major x (embedding B input)
    xT_old = np.zeros((IN_DIM + 1, NCORES * npc), np.float32)
    for c in range(NCORES):
        s, n = int(p.core_start[c]), int(p.n_c[c])
        xT_old[:IN_DIM, c * npc: c * npc + n] = x[s: s + n].T
    xT_old[IN_DIM, :] = 1.0
    # iter-0 gather table: padded-x rows [x | 1 | 0...] in QUARTER-MAJOR
    # row order (col IN_DIM = 1 so aggregation also yields the degree row;
    # emb is applied post-aggregation by linearity: Agg(h0)=emb(Agg([x;1])))
    cm = np.arange(NCORES * npc)
    cc_, ll_ = cm // npc, cm % npc
    qm = (ll_ // qs) * p.ss + cc_ * qs + (ll_ % qs)
    xrows = np.zeros((NCORES * npc, HID), ml_dtypes.bfloat16)
    xrows[qm, : IN_DIM + 1] = xT_old.T.astype(ml_dtypes.bfloat16)

    embWT_aug = np.zeros((IN_DIM + 1, HID), np.float32)
    embWT_aug[:IN_DIM] = emb_w.T
    embWT_aug[IN_DIM] = emb_b

    iota = (np.arange(128, dtype=np.float32)[None, None, :] * np.ones(
        (128, 1, 1), np.float32)).astype(ml_dtypes.bfloat16)

    shared = {
        "embWT": np.ascontiguousarray(embWT_aug),
        "xrows": xrows,
        "iota": np.ascontiguousarray(iota),
        "aWT": np.ascontiguousarray(aW.T.astype(np.float32)),
        "linWT": np.ascontiguousarray(lin_w.T).astype(ml_dtypes.bfloat16),
        "asymB": asym_b.astype(np.float32).reshape(128, 1),
        "ident": np.eye(128, dtype=np.float32),
        "r1wt_add": np.ascontiguousarray(r1_w[:, 0:128].T.astype(np.float32)),
        "r1wt_max": np.ascontiguousarray(r1_w[:, 128:256].T.astype(np.float32)),
        "r1wt_mean": np.ascontiguousarray(r1_w[:, 256:384].T.astype(np.float32)),
        "r1b_a": r1_b[0:128].astype(np.float32).reshape(128, 1),
        "r1b_b": r1_b[128:192].astype(np.float32).reshape(64, 1),
        "r2wt_a": np.ascontiguousarray(r2_w[:, 0:128].T.astype(np.float32)),
        "r2wt_b": np.ascontiguousarray(r2_w[:, 128:192].T.astype(np.float32)),
        "r2b": np.tile(r2_b.astype(np.float32).reshape(1, 16), (GPC, 1)),
    }
    in_maps = []
    for c in range(NCORES):
        m = dict(shared)
        m["xTloc"] = np.ascontiguousarray(xT_old[:, c * npc: (c + 1) * npc])
        m["msgidx"] = p.msgidx[c]
        m["slots"] = p.slots[c]
        m["poolidx"] = p.poolidx[c]
        m["poolhot"] = np.ascontiguousarray(
            p.poolhot[c].reshape(128, p.nblk * GPC))
        m["invcnt"] = p.invcnt[c]
        in_maps.append(m)
    return in_maps


# ---------------------------------------------------------------- device

def build_program(p, stage=99):
    nc = bacc.Bacc("TRN2", num_devices=NCORES, num_swdge_queues=NQ,
                   debug=False)
    npc = p.npc_pad
    nblk = p.nblk
    qrows = npc // NQ  # staging rows per quarter

    # I/O
    d_xrows = nc.dram_tensor("xrows", [NCORES * npc, HID], BF16,
                             kind="ExternalInput")
    d_xTloc = nc.dram_tensor("xTloc", [IN_DIM + 1, npc], F32,
                             kind="ExternalInput")
    d_embWT = nc.dram_tensor("embWT", [IN_DIM + 1, HID], F32,
                             kind="ExternalInput")
    d_iota = nc.dram_tensor("iota", [128, 1, 128], BF16, kind="ExternalInput")
    d_aWT = nc.dram_tensor("aWT", [HID, HID], F32, kind="ExternalInput")
    d_linWT = nc.dram_tensor("linWT", [HID, HID], BF16, kind="ExternalInput")
    d_asymB = nc.dram_tensor("asymB", [HID, 1], F32, kind="ExternalInput")
    d_ident = nc.dram_tensor("ident", [128, 128], F32, kind="ExternalInput")
    d_msgidx = nc.dram_tensor("msgidx", [128, p.tok_tot // 16], I16,
                              kind="ExternalInput")
    d_slots = nc.dram_tensor("slots", [128, p.c_tot, 1], BF16,
                             kind="ExternalInput")
    d_poolidx = nc.dram_tensor("poolidx", [128, p.pool_tok // 16], I16,
                               kind="ExternalInput")
    d_poolhot = nc.dram_tensor("poolhot", [128, nblk * GPC], BF16,
                               kind="ExternalInput")
    d_invcnt = nc.dram_tensor("invcnt", [128, GPC], F32, kind="ExternalInput")
    d_r1wt = [nc.dram_tensor(nm, [128, 192], F32, kind="ExternalInput")
              for nm in ("r1wt_add", "r1wt_max", "r1wt_mean")]
    d_r1b_a = nc.dram_tensor("r1b_a", [128, 1], F32, kind="ExternalInput")
    d_r1b_b = nc.dram_tensor("r1b_b", [64, 1], F32, kind="ExternalInput")
    d_r2wt_a = nc.dram_tensor("r2wt_a", [128, 16], F32, kind="ExternalInput")
    d_r2wt_b = nc.dram_tensor("r2wt_b", [64, 16], F32, kind="ExternalInput")
    d_r2b = nc.dram_tensor("r2b", [GPC, 16], F32, kind="ExternalInput")
    d_out = nc.dram_tensor("out", [GPC, OUT_DIM], F32, kind="ExternalOutput")
    d_dbg = nc.dram_tensor("dbg", [128, 1024], F32, kind="ExternalOutput")

    # internal DRAM
    d_stage = nc.dram_tensor("stage", [npc + 128, HID], BF16, kind="Internal")
    d_tab = [nc.dram_tensor(f"table{t}", [NCORES * npc, HID], BF16,
                            kind="Internal", addr_space="Shared")
             for t in range(2)]

    Tanh = mybir.ActivationFunctionType.Tanh
    Ident = mybir.ActivationFunctionType.Identity
    ADD = mybir.AluOpType.add
    MAX = mybir.AluOpType.max
    MULT = mybir.AluOpType.mult
    ISEQ = mybir.AluOpType.is_equal
    X = mybir.AxisListType.X
    rg = [list(range(NCORES))]

    with tile.TileContext(nc) as tc:
        with tc.tile_pool(name="const", bufs=1) as cst, \
             tc.tile_pool(name="state", bufs=1) as st, \
             tc.tile_pool(name="msg", bufs=MSG_BUFS) as msgp, \
             tc.tile_pool(name="pmsg", bufs=2) as pmsgp, \
             tc.tile_pool(name="oh", bufs=2) as ohp, \
             tc.tile_pool(name="idx", bufs=2) as idxp, \
             tc.tile_pool(name="wrk", bufs=4) as wrk, \
             tc.tile_pool(name="xt", bufs=2) as xtp, \
             tc.tile_pool(name="stg", bufs=2) as stgp, \
             tc.tile_pool(name="psA", bufs=GB, space="PSUM") as psA, \
             tc.tile_pool(name="psB", bufs=2, space="PSUM") as psB, \
             tc.tile_pool(name="psT", bufs=1, space="PSUM") as psT, \
             tc.tile_pool(name="psP", bufs=1, space="PSUM") as psP:

            # ---- load constants
            embWT = cst.tile([IN_DIM + 1, HID], F32)
            iota = cst.tile([128, 1, 128], BF16)
            aWT = cst.tile([HID, HID], F32)
            linWT = cst.tile([HID, HID], BF16)
            asymB = cst.tile([HID, 1], F32)
            ident = cst.tile([128, 128], F32)
            poolhot = cst.tile([128, nblk * GPC], BF16)
            invcnt = cst.tile([128, GPC], F32)
            slots = cst.tile([128, p.c_tot, 1], BF16)
            nc.sync.dma_start(embWT[:], d_embWT[:])
            nc.sync.dma_start(iota[:], d_iota[:])
            nc.sync.dma_start(aWT[:], d_aWT[:])
            nc.sync.dma_start(linWT[:], d_linWT[:])
            nc.sync.dma_start(asymB[:], d_asymB[:])
            nc.sync.dma_start(ident[:], d_ident[:])
            nc.sync.dma_start(poolhot[:], d_poolhot[:])
            nc.sync.dma_start(invcnt[:], d_invcnt[:])
            nc.scalar.dma_start(slots[:], d_slots[:])

            HT = st.tile([HID, npc], F32)
            ACC = st.tile([HID, npc], BF16)
            idx_res = st.tile([128, p.tok_tot // 16], I16)
            nc.scalar.dma_start(idx_res[:], d_msgidx[:])

            # -inf pad row for max-pool gather
            minf = wrk.tile([1, HID], BF16, tag="minf")
            nc.vector.memset(minf[:], NEG_BIG)
            nc.sync.dma_start(d_stage[npc: npc + 1, :], minf[:])

            # (no embedding-A table build: iter 0 gathers padded-x rows
            # directly and applies emb post-aggregation, by linearity)
            embWT16 = cst.tile([IN_DIM + 1, HID], BF16)
            nc.scalar.copy(embWT16[:], embWT[:])

            # ---- embedding B: local feature-major HT
            for j in range(npc // EMB_CHUNK):
                xt = xtp.tile([IN_DIM + 1, EMB_CHUNK], F32, tag="xtl")
                nc.scalar.dma_start(
                    xt[:], d_xTloc[:, j * EMB_CHUNK: (j + 1) * EMB_CHUNK])
                pse = psB.tile([HID, EMB_CHUNK], F32, tag="conv")
                nc.tensor.matmul(pse[:], embWT[:], xt[:], start=True,
                                 stop=True)
                nc.vector.tensor_copy(
                    HT[:, j * EMB_CHUNK: (j + 1) * EMB_CHUNK], pse[:])

            # ---------------- main iterations
            pool_ps = None
            n_iters = NUM_ITERS if stage >= 4 else (1 if stage == 3 else 0)
            for it in range(n_iters):
                last = it == n_iters - 1
                if last:
                    pool_ps = psP.tile([HID, GPC], F32, tag="poolps")
                src_tab = d_xrows if it == 0 else d_tab[it % 2]
                # gather emission order: queue-rotated subcalls; the first
                # DEFER_K3 groups' stream-3 subcalls are pushed behind the
                # other streams so the Pool engine keeps gathering while the
                # previous iteration's last-quarter AllGather lands
                defer = DEFER_K3 if it > 0 else 0
                gorder = []
                stash = []
                for G, (_, _, subs) in enumerate(p.sched):
                    for sub in subs:
                        if G < defer and sub[0] == NQ - 1:
                            stash.append((G,) + sub)
                        else:
                            gorder.append((G,) + sub)
                    if G == defer - 1:
                        gorder += stash
                        stash = []
                last_pos = {}
                for i, e in enumerate(gorder):
                    last_pos[e[0]] = i
                msub_all = [dict() for _ in range(p.ngroups)]
                gpos = 0

                for G, (oh_list, call_list, subs) in enumerate(p.sched):
                    while gpos <= last_pos[G]:
                        (eg, k, sc0, nch) = gorder[gpos]
                        gpos += 1
                        mt = msgp.tile([128, SUBCH, HID], BF16, tag="msg")
                        nc.gpsimd.dma_gather(
                            mt[:, :nch, :],
                            src_tab[k * p.ss: (k + 1) * p.ss, :],
                            idx_res[:, sc0 * 8: (sc0 + nch) * 8],
                            nch * 128, nch * 128, HID,
                            single_packet=True, queue_num=k)
                        for j in range(nch):
                            msub_all[eg][sc0 + j] = (mt, j)
                    msub = msub_all[G]

                    # one-hot per stream over the group's contiguous cols
                    oht = {}
                    for (k, oc0, oncols) in oh_list:
                        oh = ohp.tile([128, p.max_ohcols, 128], BF16,
                                      tag="oh")
                        nc.vector.tensor_tensor(
                            oh[:, :oncols, :],
                            slots[:, oc0: oc0 + oncols, :]
                            .to_broadcast([128, oncols, 128]),
                            iota[:].to_broadcast([128, oncols, 128]),
                            ISEQ)
                        oht[oc0] = (oh, oncols)

                    def oh_slice(c0, j):
                        for oc0, (oh, oncols) in oht.items():
                            if oc0 <= c0 and c0 - oc0 < oncols:
                                return oh, c0 - oc0 + j
                        raise AssertionError

                    # aggregation matmuls (stream order: k=0 starts)
                    blocks = p.groups[G]
                    psb = {}
                    for b in blocks:
                        agg_ps = psA.tile([HID, 128], F32, tag="agg")
                        psb[b] = agg_ps
                    for (k, b, c0, cc) in call_list:
                        for j in range(cc):
                            oh, col = oh_slice(c0, j)
                            mt, mcol = msub[c0 + j]
                            nc.tensor.matmul(
                                psb[b][:], mt[:, mcol, :], oh[:, col, :],
                                start=(k == 0 and j == 0),
                                stop=(k == NQ - 1 and j == cc - 1),
                                skip_group_check=True)

                    # fused phase B
                    stg = stgp.tile([128, GB * HID], BF16, tag="stage")
                    for bi, b in enumerate(blocks):
                        sl = ACC[:, b * 128: (b + 1) * 128]
                        if it == 0:
                            # psb holds Agg([x|1]); apply emb now (deg row
                            # via the ones column folds in the bias term)
                            ax = wrk.tile([IN_DIM + 1, 128], BF16,
                                          tag="aggx")
                            nc.scalar.copy(ax[:], psb[b][0: IN_DIM + 1, :])
                            psE = psB.tile([HID, 128], F32, tag="conv")
                            nc.tensor.matmul(psE[:], embWT16[:], ax[:],
                                             start=True, stop=True)
                            nc.scalar.copy(sl, psE[:])
                        else:
                            nc.scalar.copy(sl, psb[b][:])
                        ps2 = psB.tile([HID, 128], F32, tag="conv")
                        nc.tensor.matmul(ps2[:], aWT[:],
                                         HT[:, b * 128: (b + 1) * 128],
                                         start=True, stop=False)
                        nc.tensor.matmul(ps2[:], linWT[:], sl,
                                         start=False, stop=True)
                        th = wrk.tile([HID, 128], F32, tag="tanh")
                        nc.scalar.activation(th[:], ps2[:], Tanh,
                                             bias=asymB[:])
                        hsl = HT[:, b * 128: (b + 1) * 128]
                        nc.vector.scalar_tensor_tensor(hsl, th[:], EPS, hsl,
                                                       MULT, ADD)
                        trp = psT.tile([128, HID], F32, tag="tr")
                        nc.tensor.transpose(trp[:], hsl, ident[:])
                        nc.scalar.copy(
                            stg[:, bi * HID: (bi + 1) * HID], trp[:])
                        if last:
                            nc.tensor.matmul(
                                pool_ps[:], stg[:, bi * HID: (bi + 1) * HID],
                                poolhot[:, b * GPC: (b + 1) * GPC],
                                start=(b == 0), stop=(b == nblk - 1),
                                skip_group_check=True)
                    b0 = blocks[0]
                    nbG = len(blocks)
                    nc.sync.dma_start(
                        d_stage[b0 * 128: b0 * 128 + nbG * 128, :]
                        .rearrange("(a p) f -> p a f", p=128),
                        stg[:, : nbG * HID].rearrange(
                            "p (a f) -> p a f", f=HID))

                    # quarter q fully staged -> AllGather its table stripe
                    # into the NEXT iteration's table, overlapping the
                    # collective with the remaining groups' gathers (which
                    # read the CURRENT table copy).
                    if not last and G in p.q_last_group:
                        q = p.q_last_group.index(G)
                        nc.gpsimd.collective_compute(
                            "AllGather", mybir.AluOpType.bypass,
                            replica_groups=rg,
                            ins=[d_stage[q * p.qs: (q + 1) * p.qs, :].opt()],
                            outs=[d_tab[(it + 1) % 2]
                                  [q * p.ss: (q + 1) * p.ss, :].opt()])

            # ---------------- debug dumps for staged runs
            if stage < 6:
                w = min(512, npc)
                dbg_t = wrk.tile([128, 1024], F32, tag="dbg")
                nc.vector.memset(dbg_t[:], 0.0)
                if stage >= 3 and n_iters > 0:
                    nc.vector.tensor_copy(dbg_t[:, 0:w], ACC[:, 0:w])
                    nc.vector.tensor_copy(dbg_t[:, 512:512 + w], HT[:, 0:w])
                else:
                    nc.vector.tensor_copy(dbg_t[:, 0:w], HT[:, 0:w])
                nc.sync.dma_start(d_dbg[:], dbg_t[:])

            if stage >= 6:
                # ---------------- pooling
                poolsum = wrk.tile([HID, GPC], F32, tag="psum_sb")
                nc.vector.tensor_copy(poolsum[:], pool_ps[:])
                poolmean = wrk.tile([HID, GPC], F32, tag="pmean_sb")
                nc.vector.tensor_tensor(poolmean[:], poolsum[:], invcnt[:],
                                        MULT)
                poolmax = wrk.tile([HID, GPC], F32, tag="pmax_sb")

                cols_per_g = p.k_pool // 128
                for (g0, ng) in p.pool_calls:
                    ncols = ng * cols_per_g
                    ntok = ncols * 128
                    t0 = g0 * p.k_pool
                    idxt = idxp.tile([128, POOL_COLS * 8], I16, tag="idx")
                    nc.sync.dma_start(
                        idxt[:, : ncols * 8],
                        d_poolidx[:, t0 // 16: t0 // 16 + ncols * 8])
                    gat = pmsgp.tile([128, POOL_COLS, HID], BF16, tag="pmsg")
                    nc.gpsimd.dma_gather(
                        gat[:, :ncols, :], d_stage[:, :],
                        idxt[:, : ncols * 8], ntok, ntok, HID,
                        single_packet=False, queue_num=(g0 // 2) % NQ)
                    for j in range(ng):
                        g = g0 + j
                        part = wrk.tile([128, HID], F32, tag="mpart")
                        nc.vector.tensor_reduce(
                            part[:],
                            gat[:, j * cols_per_g: (j + 1) * cols_per_g, :]
                            .rearrange("p c f -> p f c"),
                            X, MAX)
                        trp = psT.tile([128, HID], F32, tag="tr")
                        nc.tensor.transpose(trp[:], part[:], ident[:])
                        nc.vector.tensor_reduce(
                            poolmax[:, g: g + 1], trp[:], X, MAX)

                # ---------------- readout MLP
                r1wt = []
                for d in d_r1wt:
                    t = cst.tile([128, 192], F32)
                    nc.sync.dma_start(t[:], d[:])
                    r1wt.append(t)
                r1b_a = cst.tile([128, 1], F32)
                r1b_b = cst.tile([64, 1], F32)
                r2wt_a = cst.tile([128, 16], F32)
                r2wt_b = cst.tile([64, 16], F32)
                r2bb = cst.tile([GPC, 16], F32)
                nc.sync.dma_start(r1b_a[:], d_r1b_a[:])
                nc.sync.dma_start(r1b_b[:], d_r1b_b[:])
                nc.sync.dma_start(r2wt_a[:], d_r2wt_a[:])
                nc.sync.dma_start(r2wt_b[:], d_r2wt_b[:])
                nc.sync.dma_start(r2bb[:], d_r2b[:])

                g1 = []
                for (m0, msz, bt) in ((0, 128, r1b_a), (128, 64, r1b_b)):
                    psr = psB.tile([msz, GPC], F32, tag="conv")
                    for wi, src_t in ((0, poolsum), (1, poolmax),
                                      (2, poolmean)):
                        nc.tensor.matmul(psr[:], r1wt[wi][:, m0: m0 + msz],
                                         src_t[:], start=(wi == 0),
                                         stop=(wi == 2))
                    gt = wrk.tile([msz, GPC], F32, tag=f"g1_{m0}")
                    nc.scalar.activation(gt[:], psr[:], Ident, bias=bt[:])
                    nc.vector.scalar_tensor_tensor(gt[:], gt[:], 0.01, gt[:],
                                                   MULT, MAX)
                    g1.append(gt)

                ps2a = psB.tile([GPC, OUT_DIM], F32, tag="conv")
                nc.tensor.matmul(ps2a[:], g1[0][:, :], r2wt_a[:],
                                 start=True, stop=True)
                ps2b = psT.tile([GPC, OUT_DIM], F32, tag="tr")
                nc.tensor.matmul(ps2b[:], g1[1][:, :], r2wt_b[:],
                                 start=True, stop=True)
                t2a = wrk.tile([GPC, OUT_DIM], F32, tag="t2a")
                nc.scalar.copy(t2a[:], ps2a[:])
                t2b = wrk.tile([GPC, OUT_DIM], F32, tag="t2b")
                nc.vector.tensor_tensor(t2b[:], ps2b[:], t2a[:], ADD)
                outt = wrk.tile([GPC, OUT_DIM], F32, tag="outt")
                nc.vector.tensor_tensor(outt[:], t2b[:], r2bb[:], ADD)
                nc.vector.scalar_tensor_tensor(outt[:], outt[:], 0.01,
                                               outt[:], MULT, MAX)
                nc.sync.dma_start(d_out[:], outt[:])

    nc.compile()
    return nc


# ---------------------------------------------------------------- entry

_CACHE = {}


def _run(inputs, trace=False, stage=99):
    x = np.asarray(inputs["x"], np.float32)
    edge_index = np.asarray(inputs["edge_index"])
    batch = np.asarray(inputs["batch"])
    plan_key = (edge_index.tobytes(), batch.tobytes(), stage)
    key = hash(plan_key)
    if key in _CACHE:
        p, nc = _CACHE[key]
    else:
        p = build_plan(edge_index, batch)
        nc = build_program(p, stage=stage)
        _CACHE[key] = (p, nc)

    in_maps = prepare_inputs(
        p, x,
        np.asarray(inputs["emb_w"], np.float32),
        np.asarray(inputs["emb_b"], np.float32),
        np.asarray(inputs["W"], np.float32),
        np.asarray(inputs["asym_b"], np.float32),
        np.asarray(inputs["lin_w"], np.float32),
        np.asarray(inputs["r1_w"], np.float32),
        np.asarray(inputs["r1_b"], np.float32),
        np.asarray(inputs["r2_w"], np.float32),
        np.asarray(inputs["r2_b"], np.float32),
    )
    res = run_bass_kernel_spmd(nc, in_maps, core_ids=list(range(NCORES)),
                               trace=trace)
    out = np.concatenate([res.results[c]["out"] for c in range(NCORES)], 0)
    return out.astype(np.float32), res


def kernel(**inputs):
    out, _ = _run(inputs, trace=False)
    return out



# revision 47
# speedup vs baseline: 2.8007x; 1.0381x over previous
"""ADGN (antisymmetric DGN) message-passing GNN on 8 TRN2 NeuronCores.

Strategy (self-contained, hardcoded for the nn_ADGN problem):
  - Nodes sharded at GRAPH boundaries: core c owns graphs [16c, 16c+16).
    Per-core node counts padded to uniform NPC_PAD (one SPMD program).
  - State HT [128 feat, NPC_PAD] f32 in SBUF; replicated bf16 h table in
    DRAM, QUARTER-MAJOR layout: row = q*2*npc + core*(npc/4) + (local %
    npc/4), q = local quarter. Two copies (A/B) alternate per iteration;
    the 4 quarter-stripes are rebuilt by 4 small AllGathers that fire as
    soon as each quarter's staging rows are written, overlapping the
    collective with the remaining sweep.
  - Iteration = fused sweep over dst-block GROUPS (GB=4 blocks, grouped
    within a table quarter):
      * one dma_gather call per (stream k, dst block b) bucket on SWDGE
        queue k; per-core token counts are exact - pad tokens carry
        index -1 which the Q7 desc-gen trims (no descriptors, no DMA).
      * one-hot tiles generated ON-CHIP, one DVE tensor_tensor is_equal
        per (group, stream) over the contiguous column range:
        oh[p, c, j] = (slot[p, c] == iota[j]).
      * per dst block: PSUM accumulates msg.T @ onehot over all streams'
        chunks; then fused phase B: conv = aW.T@HT_b + lin.T@ACC_b,
        HT_b += eps*tanh(conv+bias), PE transpose -> bf16 staging ->
        DRAM stage (+ pool-sum matmuls in the last iteration).
  - Iter 0 table comes from a full-table node-major embedding computed
    locally on every core (bias via augmented ones-row): no initial
    AllGather; 3 iterations x 4 quarter-AllGathers total.
  - Pooling: sum via batch-one-hot matmuls on staging tiles; max via
    -inf-padded dma_gather + two-level reduce; mean = sum * 1/cnt.
  - Readout MLP on-device per core ([16 graphs, 16]); host concatenates.
"""

import sys

if "/opt/trn_rl_repo" not in sys.path:
    sys.path.insert(0, "/opt/trn_rl_repo")

import numpy as np
import ml_dtypes

import concourse.bacc as bacc
import concourse.bass as bass
import concourse.mybir as mybir
import concourse.tile as tile
from concourse.bass_utils import run_bass_kernel_spmd

F32 = mybir.dt.float32
BF16 = mybir.dt.bfloat16
I16 = mybir.dt.int16

NCORES = 8
N_GRAPHS = 128
GPC = N_GRAPHS // NCORES  # graphs per core
IN_DIM, HID, OUT_DIM = 64, 128, 16
NUM_ITERS = 4
GAMMA, EPS = 0.1, 0.1
NQ = 4                  # SWDGE queues == streams == table quarters
GB = 4                  # dst blocks per group
EMB_CHUNK = 512
NEG_BIG = -1.0e30
POOL_COLS = 14          # max gather-call size for POOLING calls
MSG_BUFS = 20
SUBCH = 8              # chunks per gather subcall
DEFER_K3 = 0            # groups whose stream-3 subcalls are deferred at the
                        # iteration start (hides the last quarter AllGather)


def _ceil_to(x, m):
    return -(-x // m) * m


# ---------------------------------------------------------------- host plan

class Plan:
    pass


def build_plan(edge_index, batch):
    """Host-side preprocessing: shared chunk schedule + per-core arrays."""
    p = Plan()
    n_nodes = batch.shape[0]
    src = np.asarray(edge_index[0], dtype=np.int64)
    dst = np.asarray(edge_index[1], dtype=np.int64)
    batch = np.asarray(batch, dtype=np.int64)
    assert (np.diff(batch) >= 0).all(), "batch must be sorted"

    # graph -> core, node -> core
    graph_start = np.searchsorted(batch, np.arange(N_GRAPHS + 1))  # [129]
    core_start = graph_start[:: GPC]  # [9]
    n_c = np.diff(core_start)
    npc_pad = int(_ceil_to(max(int(n_c.max()), EMB_CHUNK), EMB_CHUNK))
    assert 2 * npc_pad <= 32767
    p.npc_pad = npc_pad
    p.nblk = npc_pad // 128
    p.qs = npc_pad // NQ          # local rows per table quarter
    p.ss = NCORES * p.qs          # table-quarter stripe rows (= 2*npc)
    p.nbq = p.nblk // NQ          # dst blocks per quarter
    p.core_start = core_start
    p.n_c = n_c
    nblk = p.nblk

    node_core = (batch // GPC).astype(np.int64)
    node_local = np.arange(n_nodes, dtype=np.int64) - core_start[node_core]
    # QUARTER-MAJOR shared-table row (stripe = local quarter, then core):
    # lets each quarter's AllGather fire as soon as that quarter is staged
    table_row = ((node_local // p.qs) * p.ss + node_core * p.qs
                 + node_local % p.qs)

    # ---- edge grouping in (quarter, group, k, b) order
    e_tr = table_row[src]
    e_k = e_tr // p.ss                      # stream = src quarter
    e_gidx = (e_tr - e_k * p.ss).astype(np.int16)
    e_core = node_core[dst]
    e_ld = node_local[dst]
    e_b = e_ld // 128
    e_slot = (e_ld % 128).astype(np.int64)

    # schedule: quarters -> groups of GB blocks -> streams -> blocks
    groups = []          # list of (list of blocks)
    for q in range(NQ):
        b0 = q * p.nbq
        for g0 in range(0, p.nbq, GB):
            groups.append(list(range(b0 + g0, b0 + min(g0 + GB, p.nbq))))
    p.groups = groups
    p.ngroups = len(groups)
    p.q_last_group = [max(i for i, g in enumerate(groups)
                          if g[0] // p.nbq == q) for q in range(NQ)]

    # bucket index per (group, k, b-within-group)
    bucket_of = np.full((NQ, nblk), -1, np.int64)   # [k, b] -> bucket id
    bid = 0
    for g in groups:
        for k in range(NQ):
            for b in g:
                bucket_of[k, b] = bid
                bid += 1
    n_bkt = bid
    e_bkt = bucket_of[e_k, e_b]

    key = e_core * n_bkt + e_bkt
    order = np.argsort(key * np.int64(40000) + e_gidx, kind="stable")
    key_s = key[order]
    counts = np.bincount(key_s, minlength=NCORES * n_bkt)
    counts = counts.reshape(NCORES, n_bkt)

    chunk_counts = (-(-counts // 128)).max(axis=0)  # [n_bkt]
    # every block needs >=1 chunk at k=0 so the PSUM start flag exists
    for b in range(nblk):
        j = bucket_of[0, b]
        chunk_counts[j] = max(chunk_counts[j], 1)
    p.chunk_counts = chunk_counts
    p.ccmax = int(chunk_counts.max())
    col_off = np.zeros_like(chunk_counts)
    col_off[1:] = np.cumsum(chunk_counts)[:-1]
    p.col_off = col_off
    p.c_tot = int(chunk_counts.sum())

    # ---- per-group schedule: gather subcalls + one-hot slices
    # p.sched[G] = (oh_list, call_list, subs)
    #   oh_list: per stream k with cols: (k, oh_c0, oh_ncols)
    #   call_list: (k, b, c0, cc) per bucket with cc > 0
    #   subs: gather subcalls (k, sc0, nch), <=SUBCH chunks each, issued
    #         sub-major / stream-minor so the 4 SWDGE queues drain in
    #         parallel (each subcall fits the 1024-desc ring carveout).
    p.sched = []
    p.max_ohcols = 0
    for gi, g in enumerate(groups):
        oh_list = []
        call_list = []
        for k in range(NQ):
            kc0 = None
            kcols = 0
            for b in g:
                j = bucket_of[k, b]
                cc = int(chunk_counts[j])
                if cc == 0:
                    continue
                c0 = int(col_off[j])
                if kc0 is None:
                    kc0 = c0
                assert c0 == kc0 + kcols
                call_list.append((k, b, c0, cc))
                kcols += cc
            if kc0 is not None:
                oh_list.append((k, kc0, kcols))
                p.max_ohcols = max(p.max_ohcols, kcols)
        subs = []
        if oh_list:
            maxs = max(-(-kcols // SUBCH) for (_, _, kcols) in oh_list)
            for s in range(maxs):
                for (k, kc0, kcols) in oh_list:
                    if s * SUBCH < kcols:
                        subs.append((k, kc0 + s * SUBCH,
                                     min(SUBCH, kcols - s * SUBCH)))
        p.sched.append((oh_list, call_list, subs))

    # ---- per-core token data (pads: idx -1 / slot -1)
    tok_tot = p.c_tot * 128
    p.tok_tot = tok_tot
    seg_start = np.zeros(NCORES * n_bkt + 1, np.int64)
    seg_start[1:] = np.cumsum(counts.reshape(-1))
    within = np.arange(len(order), dtype=np.int64) - seg_start[key_s]
    tokpos = col_off[key_s % n_bkt] * 128 + within

    p.msgidx = []
    p.slots = []
    for c in range(NCORES):
        mask = (key_s // n_bkt) == c
        tp = tokpos[mask]
        gi = e_gidx[order][mask]
        sl = e_slot[order][mask]
        arr16 = np.zeros((16, tok_tot // 16), np.int16)
        arr16[tp % 16, tp // 16] = gi
        p.msgidx.append(np.tile(arr16, (8, 1)))
        slots = np.full((128, p.c_tot, 1), -1.0, ml_dtypes.bfloat16)
        slots[tp % 128, tp // 128, 0] = sl.astype(ml_dtypes.bfloat16)
        p.slots.append(slots)

    # ---- pooling
    gsz = np.diff(graph_start)
    p.k_pool = int(_ceil_to(max(int(gsz.max()), 128), 128))
    pool_cols_per_graph = p.k_pool // 128
    graphs_per_call = max(1, POOL_COLS // pool_cols_per_graph)
    p.pool_calls = []
    g = 0
    while g < GPC:
        ng = min(graphs_per_call, GPC - g)
        p.pool_calls.append((g, ng))
        g += ng
    pool_tok = GPC * p.k_pool
    p.pool_tok = pool_tok
    # staging-row upper bound per pool call (over cores): lets each pool
    # gather depend only on the staging writes that can feed it
    p.pool_rmax = [
        max(int(graph_start[c * GPC + g0 + ng] - core_start[c])
            for c in range(NCORES))
        for (g0, ng) in p.pool_calls
    ]

    p.poolidx = []
    p.poolhot = []
    p.invcnt = []
    for c in range(NCORES):
        idx = np.full(pool_tok, npc_pad, np.int64)  # default: -inf pad row
        for j in range(GPC):
            g_id = c * GPC + j
            s = graph_start[g_id] - core_start[c]
            e = graph_start[g_id + 1] - core_start[c]
            idx[j * p.k_pool: j * p.k_pool + (e - s)] = np.arange(s, e)
        arr16 = np.zeros((16, pool_tok // 16), np.int16)
        t = np.arange(pool_tok)
        arr16[t % 16, t // 16] = idx.astype(np.int16)
        p.poolidx.append(np.tile(arr16, (8, 1)))

        ph = np.zeros((128, p.nblk, GPC), ml_dtypes.bfloat16)
        ln = np.arange(int(n_c[c]), dtype=np.int64)
        gslot = batch[core_start[c]: core_start[c + 1]] - c * GPC
        ph[ln % 128, ln // 128, gslot] = 1.0
        p.poolhot.append(ph)

        cnt = gsz[c * GPC: (c + 1) * GPC].astype(np.float32)
        inv = 1.0 / np.maximum(cnt, 1.0)
        p.invcnt.append(np.tile(inv[None, :], (128, 1)).astype(np.float32))

    return p


def prepare_inputs(p, x, emb_w, emb_b, W, asym_b, lin_w, r1_w, r1_b, r2_w,
                   r2_b):
    """Build per-core in_maps."""
    aW = W - W.T - GAMMA * np.eye(HID, dtype=np.float32)
    npc = p.npc_pad
    qs = p.qs

    # local feature-major x (embedding B input)
    xT_old = np.zeros((IN_DIM + 1, NCORES * npc), np.float32)
    for c in range(NCORES):
        s, n = int(p.core_start[c]), int(p.n_c[c])
        xT_old[:IN_DIM, c * npc: c * npc + n] = x[s: s + n].T
    xT_old[IN_DIM, :] = 1.0
    # iter-0 gather table: padded-x rows [x | 1 | 0...] in QUARTER-MAJOR
    # row order (col IN_DIM = 1 so aggregation also yields the degree row;
    # emb is applied post-aggregation by linearity: Agg(h0)=emb(Agg([x;1])))
    cm = np.arange(NCORES * npc)
    cc_, ll_ = cm // npc, cm % npc
    qm = (ll_ // qs) * p.ss + cc_ * qs + (ll_ % qs)
    xrows = np.zeros((NCORES * npc, HID), ml_dtypes.bfloat16)
    xrows[qm, : IN_DIM + 1] = xT_old.T.astype(ml_dtypes.bfloat16)

    embWT_aug = np.zeros((IN_DIM + 1, HID), np.float32)
    embWT_aug[:IN_DIM] = emb_w.T
    embWT_aug[IN_DIM] = emb_b

    iota = (np.arange(128, dtype=np.float32)[None, None, :] * np.ones(
        (128, 1, 1), np.float32)).astype(ml_dtypes.bfloat16)

    shared = {
        "embWT": np.ascontiguousarray(embWT_aug),
        "xrows": xrows,
        "iota": np.ascontiguousarray(iota),
        "aWT": np.ascontiguousarray(aW.T.astype(np.float32)),
        "linWT": np.ascontiguousarray(lin_w.T).astype(ml_dtypes.bfloat16),
        "asymB": asym_b.astype(np.float32).reshape(128, 1),
        "ident": np.eye(128, dtype=np.float32),
        "r1wt_add": np.ascontiguousarray(r1_w[:, 0:128].T.astype(np.float32)),
        "r1wt_max": np.ascontiguousarray(r1_w[:, 128:256].T.astype(np.float32)),
        "r1wt_mean": np.ascontiguousarray(r1_w[:, 256:384].T.astype(np.float32)),
        "r1b_a": r1_b[0:128].astype(np.float32).reshape(128, 1),
        "r1b_b": r1_b[128:192].astype(np.float32).reshape(64, 1),
        "r2wt_a": np.ascontiguousarray(r2_w[:, 0:128].T.astype(np.float32)),
        "r2wt_b": np.ascontiguousarray(r2_w[:, 128:192].T.astype(np.float32)),
        "r2b": np.tile(r2_b.astype(np.float32).reshape(1, 16), (GPC, 1)),
    }
    in_maps = []
    for c in range(NCORES):
        m = dict(shared)
        m["xTloc"] = np.ascontiguousarray(xT_old[:, c * npc: (c + 1) * npc])
        m["msgidx"] = p.msgidx[c]
        m["slots"] = p.slots[c]
        m["poolidx"] = p.poolidx[c]
        m["poolhot"] = np.ascontiguousarray(
            p.poolhot[c].reshape(128, p.nblk * GPC))
        m["invcnt"] = p.invcnt[c]
        in_maps.append(m)
    return in_maps


# ---------------------------------------------------------------- device

def build_program(p, stage=99):
    nc = bacc.Bacc("TRN2", num_devices=NCORES, num_swdge_queues=NQ,
                   debug=False)
    npc = p.npc_pad
    nblk = p.nblk
    qrows = npc // NQ  # staging rows per quarter

    # I/O
    d_xrows = nc.dram_tensor("xrows", [NCORES * npc, HID], BF16,
                             kind="ExternalInput")
    d_xTloc = nc.dram_tensor("xTloc", [IN_DIM + 1, npc], F32,
                             kind="ExternalInput")
    d_embWT = nc.dram_tensor("embWT", [IN_DIM + 1, HID], F32,
                             kind="ExternalInput")
    d_iota = nc.dram_tensor("iota", [128, 1, 128], BF16, kind="ExternalInput")
    d_aWT = nc.dram_tensor("aWT", [HID, HID], F32, kind="ExternalInput")
    d_linWT = nc.dram_tensor("linWT", [HID, HID], BF16, kind="ExternalInput")
    d_asymB = nc.dram_tensor("asymB", [HID, 1], F32, kind="ExternalInput")
    d_ident = nc.dram_tensor("ident", [128, 128], F32, kind="ExternalInput")
    d_msgidx = nc.dram_tensor("msgidx", [128, p.tok_tot // 16], I16,
                              kind="ExternalInput")
    d_slots = nc.dram_tensor("slots", [128, p.c_tot, 1], BF16,
                             kind="ExternalInput")
    d_poolidx = nc.dram_tensor("poolidx", [128, p.pool_tok // 16], I16,
                               kind="ExternalInput")
    d_poolhot = nc.dram_tensor("poolhot", [128, nblk * GPC], BF16,
                               kind="ExternalInput")
    d_invcnt = nc.dram_tensor("invcnt", [128, GPC], F32, kind="ExternalInput")
    d_r1wt = [nc.dram_tensor(nm, [128, 192], F32, kind="ExternalInput")
              for nm in ("r1wt_add", "r1wt_max", "r1wt_mean")]
    d_r1b_a = nc.dram_tensor("r1b_a", [128, 1], F32, kind="ExternalInput")
    d_r1b_b = nc.dram_tensor("r1b_b", [64, 1], F32, kind="ExternalInput")
    d_r2wt_a = nc.dram_tensor("r2wt_a", [128, 16], F32, kind="ExternalInput")
    d_r2wt_b = nc.dram_tensor("r2wt_b", [64, 16], F32, kind="ExternalInput")
    d_r2b = nc.dram_tensor("r2b", [GPC, 16], F32, kind="ExternalInput")
    d_out = nc.dram_tensor("out", [GPC, OUT_DIM], F32, kind="ExternalOutput")
    d_dbg = nc.dram_tensor("dbg", [128, 1024], F32, kind="ExternalOutput")

    # internal DRAM
    d_stage = nc.dram_tensor("stage", [npc + 128, HID], BF16, kind="Internal")
    d_tab = [nc.dram_tensor(f"table{t}", [NCORES * npc, HID], BF16,
                            kind="Internal", addr_space="Shared")
             for t in range(2)]

    Tanh = mybir.ActivationFunctionType.Tanh
    Ident = mybir.ActivationFunctionType.Identity
    ADD = mybir.AluOpType.add
    MAX = mybir.AluOpType.max
    MULT = mybir.AluOpType.mult
    ISEQ = mybir.AluOpType.is_equal
    X = mybir.AxisListType.X
    rg = [list(range(NCORES))]

    with tile.TileContext(nc) as tc:
        with tc.tile_pool(name="const", bufs=1) as cst, \
             tc.tile_pool(name="state", bufs=1) as st, \
             tc.tile_pool(name="msg", bufs=MSG_BUFS) as msgp, \
             tc.tile_pool(name="pmsg", bufs=2) as pmsgp, \
             tc.tile_pool(name="oh", bufs=2) as ohp, \
             tc.tile_pool(name="idx", bufs=2) as idxp, \
             tc.tile_pool(name="wrk", bufs=4) as wrk, \
             tc.tile_pool(name="xt", bufs=2) as xtp, \
             tc.tile_pool(name="stg", bufs=2) as stgp, \
             tc.tile_pool(name="psA", bufs=GB, space="PSUM") as psA, \
             tc.tile_pool(name="psB", bufs=2, space="PSUM") as psB, \
             tc.tile_pool(name="psT", bufs=1, space="PSUM") as psT, \
             tc.tile_pool(name="psP", bufs=1, space="PSUM") as psP:

            # ---- load constants
            embWT = cst.tile([IN_DIM + 1, HID], F32)
            iota = cst.tile([128, 1, 128], BF16)
            aWT = cst.tile([HID, HID], F32)
            linWT = cst.tile([HID, HID], BF16)
            asymB = cst.tile([HID, 1], F32)
            ident = cst.tile([128, 128], F32)
            poolhot = cst.tile([128, nblk * GPC], BF16)
            invcnt = cst.tile([128, GPC], F32)
            slots = cst.tile([128, p.c_tot, 1], BF16)
            nc.sync.dma_start(embWT[:], d_embWT[:])
            nc.sync.dma_start(iota[:], d_iota[:])
            nc.sync.dma_start(aWT[:], d_aWT[:])
            nc.sync.dma_start(linWT[:], d_linWT[:])
            nc.sync.dma_start(asymB[:], d_asymB[:])
            nc.sync.dma_start(ident[:], d_ident[:])
            nc.sync.dma_start(poolhot[:], d_poolhot[:])
            nc.sync.dma_start(invcnt[:], d_invcnt[:])
            nc.scalar.dma_start(slots[:], d_slots[:])

            HT = st.tile([HID, npc], F32)
            ACC = st.tile([HID, npc], BF16)
            idx_res = st.tile([128, p.tok_tot // 16], I16)
            # split the load so early gathers only wait for their slice
            idxw = p.tok_tot // 16
            for i4 in range(4):
                a, b4 = idxw * i4 // 4, idxw * (i4 + 1) // 4
                nc.scalar.dma_start(idx_res[:, a: b4], d_msgidx[:, a: b4])

            # -inf pad row for max-pool gather
            minf = wrk.tile([1, HID], BF16, tag="minf")
            nc.vector.memset(minf[:], NEG_BIG)
            nc.sync.dma_start(d_stage[npc: npc + 1, :], minf[:])

            # (no embedding-A table build: iter 0 gathers padded-x rows
            # directly and applies emb post-aggregation, by linearity)
            embWT16 = cst.tile([IN_DIM + 1, HID], BF16)
            nc.scalar.copy(embWT16[:], embWT[:])

            # readout weights loaded up front so the tail never waits
            r1wt = []
            for d in d_r1wt:
                t = cst.tile([128, 192], F32)
                nc.sync.dma_start(t[:], d[:])
                r1wt.append(t)
            r1b_a = cst.tile([128, 1], F32)
            r1b_b = cst.tile([64, 1], F32)
            r2wt_a = cst.tile([128, 16], F32)
            r2wt_b = cst.tile([64, 16], F32)
            r2bb = cst.tile([GPC, 16], F32)
            nc.sync.dma_start(r1b_a[:], d_r1b_a[:])
            nc.sync.dma_start(r1b_b[:], d_r1b_b[:])
            nc.sync.dma_start(r2wt_a[:], d_r2wt_a[:])
            nc.sync.dma_start(r2wt_b[:], d_r2wt_b[:])
            nc.sync.dma_start(r2bb[:], d_r2b[:])

            # ---- embedding B: local feature-major HT
            for j in range(npc // EMB_CHUNK):
                xt = xtp.tile([IN_DIM + 1, EMB_CHUNK], F32, tag="xtl")
                nc.scalar.dma_start(
                    xt[:], d_xTloc[:, j * EMB_CHUNK: (j + 1) * EMB_CHUNK])
                pse = psB.tile([HID, EMB_CHUNK], F32, tag="conv")
                nc.tensor.matmul(pse[:], embWT[:], xt[:], start=True,
                                 stop=True)
                nc.vector.tensor_copy(
                    HT[:, j * EMB_CHUNK: (j + 1) * EMB_CHUNK], pse[:])

            # ---------------- main iterations
            pool_ps = None
            n_iters = NUM_ITERS if stage >= 4 else (1 if stage == 3 else 0)
            for it in range(n_iters):
                last = it == n_iters - 1
                if last:
                    pool_ps = psP.tile([HID, GPC], F32, tag="poolps")
                src_tab = d_xrows if it == 0 else d_tab[it % 2]
                # gather emission order: queue-rotated subcalls; the first
                # DEFER_K3 groups' stream-3 subcalls are pushed behind the
                # other streams so the Pool engine keeps gathering while the
                # previous iteration's last-quarter AllGather lands
                defer = DEFER_K3 if it > 0 else 0
                gorder = []
                stash = []
                for G, (_, _, subs) in enumerate(p.sched):
                    for sub in subs:
                        if G < defer and sub[0] == NQ - 1:
                            stash.append((G,) + sub)
                        else:
                            gorder.append((G,) + sub)
                    if G == defer - 1:
                        gorder += stash
                        stash = []
                last_pos = {}
                for i, e in enumerate(gorder):
                    last_pos[e[0]] = i
                msub_all = [dict() for _ in range(p.ngroups)]
                gpos = 0

                for G, (oh_list, call_list, subs) in enumerate(p.sched):
                    while gpos <= last_pos[G]:
                        (eg, k, sc0, nch) = gorder[gpos]
                        gpos += 1
                        mt = msgp.tile([128, SUBCH, HID], BF16, tag="msg")
                        nc.gpsimd.dma_gather(
                            mt[:, :nch, :],
                            src_tab[k * p.ss: (k + 1) * p.ss, :],
                            idx_res[:, sc0 * 8: (sc0 + nch) * 8],
                            nch * 128, nch * 128, HID,
                            single_packet=True, queue_num=k)
                        for j in range(nch):
                            msub_all[eg][sc0 + j] = (mt, j)
                    msub = msub_all[G]

                    # one-hot per stream over the group's contiguous cols
                    oht = {}
                    for (k, oc0, oncols) in oh_list:
                        oh = ohp.tile([128, p.max_ohcols, 128], BF16,
                                      tag="oh")
                        nc.vector.tensor_tensor(
                            oh[:, :oncols, :],
                            slots[:, oc0: oc0 + oncols, :]
                            .to_broadcast([128, oncols, 128]),
                            iota[:].to_broadcast([128, oncols, 128]),
                            ISEQ)
                        oht[oc0] = (oh, oncols)

                    def oh_slice(c0, j):
                        for oc0, (oh, oncols) in oht.items():
                            if oc0 <= c0 and c0 - oc0 < oncols:
                                return oh, c0 - oc0 + j
                        raise AssertionError

                    # aggregation matmuls (stream order: k=0 starts)
                    blocks = p.groups[G]
                    psb = {}
                    for b in blocks:
                        agg_ps = psA.tile([HID, 128], F32, tag="agg")
                        psb[b] = agg_ps
                    for (k, b, c0, cc) in call_list:
                        for j in range(cc):
                            oh, col = oh_slice(c0, j)
                            mt, mcol = msub[c0 + j]
                            nc.tensor.matmul(
                                psb[b][:], mt[:, mcol, :], oh[:, col, :],
                                start=(k == 0 and j == 0),
                                stop=(k == NQ - 1 and j == cc - 1),
                                skip_group_check=True)

                    # fused phase B
                    stg = stgp.tile([128, GB * HID], BF16, tag="stage")
                    for bi, b in enumerate(blocks):
                        sl = ACC[:, b * 128: (b + 1) * 128]
                        if it == 0:
                            # psb holds Agg([x|1]); apply emb now (deg row
                            # via the ones column folds in the bias term)
                            ax = wrk.tile([IN_DIM + 1, 128], BF16,
                                          tag="aggx")
                            nc.scalar.copy(ax[:], psb[b][0: IN_DIM + 1, :])
                            psE = psB.tile([HID, 128], F32, tag="conv")
                            nc.tensor.matmul(psE[:], embWT16[:], ax[:],
                                             start=True, stop=True)
                            nc.scalar.copy(sl, psE[:])
                        else:
                            nc.scalar.copy(sl, psb[b][:])
                        ps2 = psB.tile([HID, 128], F32, tag="conv")
                        nc.tensor.matmul(ps2[:], aWT[:],
                                         HT[:, b * 128: (b + 1) * 128],
                                         start=True, stop=False)
                        nc.tensor.matmul(ps2[:], linWT[:], sl,
                                         start=False, stop=True)
                        th = wrk.tile([HID, 128], F32, tag="tanh")
                        nc.scalar.activation(th[:], ps2[:], Tanh,
                                             bias=asymB[:])
                        hsl = HT[:, b * 128: (b + 1) * 128]
                        nc.vector.scalar_tensor_tensor(hsl, th[:], EPS, hsl,
                                                       MULT, ADD)
                        trp = psT.tile([128, HID], F32, tag="tr")
                        nc.tensor.transpose(trp[:], hsl, ident[:])
                        nc.scalar.copy(
                            stg[:, bi * HID: (bi + 1) * HID], trp[:])
                        if last:
                            nc.tensor.matmul(
                                pool_ps[:], stg[:, bi * HID: (bi + 1) * HID],
                                poolhot[:, b * GPC: (b + 1) * GPC],
                                start=(b == 0), stop=(b == nblk - 1),
                                skip_group_check=True)
                    b0 = blocks[0]
                    nbG = len(blocks)
                    nc.sync.dma_start(
                        d_stage[b0 * 128: b0 * 128 + nbG * 128, :]
                        .rearrange("(a p) f -> p a f", p=128),
                        stg[:, : nbG * HID].rearrange(
                            "p (a f) -> p a f", f=HID))

                    # quarter q fully staged -> AllGather its table stripe
                    # into the NEXT iteration's table, overlapping the
                    # collective with the remaining groups' gathers (which
                    # read the CURRENT table copy).
                    if not last and G in p.q_last_group:
                        q = p.q_last_group.index(G)
                        nc.gpsimd.collective_compute(
                            "AllGather", mybir.AluOpType.bypass,
                            replica_groups=rg,
                            ins=[d_stage[q * p.qs: (q + 1) * p.qs, :].opt()],
                            outs=[d_tab[(it + 1) % 2]
                                  [q * p.ss: (q + 1) * p.ss, :].opt()])

            # ---------------- debug dumps for staged runs
            if stage < 6:
                w = min(512, npc)
                dbg_t = wrk.tile([128, 1024], F32, tag="dbg")
                nc.vector.memset(dbg_t[:], 0.0)
                if stage >= 3 and n_iters > 0:
                    nc.vector.tensor_copy(dbg_t[:, 0:w], ACC[:, 0:w])
                    nc.vector.tensor_copy(dbg_t[:, 512:512 + w], HT[:, 0:w])
                else:
                    nc.vector.tensor_copy(dbg_t[:, 0:w], HT[:, 0:w])
                nc.sync.dma_start(d_dbg[:], dbg_t[:])

            if stage >= 6:
                # ---------------- pooling
                poolsum = wrk.tile([HID, GPC], F32, tag="psum_sb")
                nc.vector.tensor_copy(poolsum[:], pool_ps[:])
                poolmean = wrk.tile([HID, GPC], F32, tag="pmean_sb")
                nc.vector.tensor_tensor(poolmean[:], poolsum[:], invcnt[:],
                                        MULT)
                poolmax = wrk.tile([HID, GPC], F32, tag="pmax_sb")

                cols_per_g = p.k_pool // 128
                for ci, (g0, ng) in enumerate(p.pool_calls):
                    ncols = ng * cols_per_g
                    ntok = ncols * 128
                    t0 = g0 * p.k_pool
                    idxt = idxp.tile([128, POOL_COLS * 8], I16, tag="idx")
                    nc.sync.dma_start(
                        idxt[:, : ncols * 8],
                        d_poolidx[:, t0 // 16: t0 // 16 + ncols * 8])
                    gat = pmsgp.tile([128, POOL_COLS, HID], BF16, tag="pmsg")
                    # range-limited src so this gather only waits for the
                    # staging writes that can feed it (the -inf pad row at
                    # npc is written once at kernel start, no hazard)
                    nc.gpsimd.dma_gather(
                        gat[:, :ncols, :],
                        d_stage[0: p.pool_rmax[ci], :],
                        idxt[:, : ncols * 8], ntok, ntok, HID,
                        single_packet=False, queue_num=(g0 // 2) % NQ)
                    for j in range(ng):
                        g = g0 + j
                        part = wrk.tile([128, HID], F32, tag="mpart")
                        nc.vector.tensor_reduce(
                            part[:],
                            gat[:, j * cols_per_g: (j + 1) * cols_per_g, :]
                            .rearrange("p c f -> p f c"),
                            X, MAX)
                        trp = psT.tile([128, HID], F32, tag="tr")
                        nc.tensor.transpose(trp[:], part[:], ident[:])
                        nc.vector.tensor_reduce(
                            poolmax[:, g: g + 1], trp[:], X, MAX)

                # ---------------- readout MLP
                g1 = []
                for (m0, msz, bt) in ((0, 128, r1b_a), (128, 64, r1b_b)):
                    psr = psB.tile([msz, GPC], F32, tag="conv")
                    for wi, src_t in ((0, poolsum), (1, poolmax),
                                      (2, poolmean)):
                        nc.tensor.matmul(psr[:], r1wt[wi][:, m0: m0 + msz],
                                         src_t[:], start=(wi == 0),
                                         stop=(wi == 2))
                    gt = wrk.tile([msz, GPC], F32, tag=f"g1_{m0}")
                    nc.scalar.activation(gt[:], psr[:], Ident, bias=bt[:])
                    nc.vector.scalar_tensor_tensor(gt[:], gt[:], 0.01, gt[:],
                                                   MULT, MAX)
                    g1.append(gt)

                ps2a = psB.tile([GPC, OUT_DIM], F32, tag="conv")
                nc.tensor.matmul(ps2a[:], g1[0][:, :], r2wt_a[:],
                                 start=True, stop=True)
                ps2b = psT.tile([GPC, OUT_DIM], F32, tag="tr")
                nc.tensor.matmul(ps2b[:], g1[1][:, :], r2wt_b[:],
                                 start=True, stop=True)
                t2a = wrk.tile([GPC, OUT_DIM], F32, tag="t2a")
                nc.scalar.copy(t2a[:], ps2a[:])
                t2b = wrk.tile([GPC, OUT_DIM], F32, tag="t2b")
                nc.vector.tensor_tensor(t2b[:], ps2b[:], t2a[:], ADD)
                outt = wrk.tile([GPC, OUT_DIM], F32, tag="outt")
                nc.vector.tensor_tensor(outt[:], t2b[:], r2bb[:], ADD)
                nc.vector.scalar_tensor_tensor(outt[:], outt[:], 0.01,
                                               outt[:], MULT, MAX)
                nc.sync.dma_start(d_out[:], outt[:])

    nc.compile()
    return nc


# ---------------------------------------------------------------- entry

_CACHE = {}


def _run(inputs, trace=False, stage=99):
    x = np.asarray(inputs["x"], np.float32)
    edge_index = np.asarray(inputs["edge_index"])
    batch = np.asarray(inputs["batch"])
    plan_key = (edge_index.tobytes(), batch.tobytes(), stage)
    key = hash(plan_key)
    if key in _CACHE:
        p, nc = _CACHE[key]
    else:
        p = build_plan(edge_index, batch)
        nc = build_program(p, stage=stage)
        _CACHE[key] = (p, nc)

    in_maps = prepare_inputs(
        p, x,
        np.asarray(inputs["emb_w"], np.float32),
        np.asarray(inputs["emb_b"], np.float32),
        np.asarray(inputs["W"], np.float32),
        np.asarray(inputs["asym_b"], np.float32),
        np.asarray(inputs["lin_w"], np.float32),
        np.asarray(inputs["r1_w"], np.float32),
        np.asarray(inputs["r1_b"], np.float32),
        np.asarray(inputs["r2_w"], np.float32),
        np.asarray(inputs["r2_b"], np.float32),
    )
    res = run_bass_kernel_spmd(nc, in_maps, core_ids=list(range(NCORES)),
                               trace=trace)
    out = np.concatenate([res.results[c]["out"] for c in range(NCORES)], 0)
    return out.astype(np.float32), res


def kernel(**inputs):
    out, _ = _run(inputs, trace=False)
    return out

